# revision 1
# baseline (speedup 1.0000x reference)
"""Trainium2 Bass kernel: fused QKV + RoPE + causal/windowed GQA attention + output proj.

Sharding: tensor-parallel by head across 8 cores. Core c owns Q-heads
4c..4c+3 and KV-group c (matching repeat_interleave grouping), plus the
512 w_o columns for those heads. Each core computes a full-shape partial
of the final output (contraction over its 512 attention-output dims);
the host sums the 8 partials. No device collectives.

Dataflow is in transposed-activation space so every matmul contracts on
the partition dim; matmul operands are bf16 (full PE rate), all
accumulation/softmax math is fp32 in PSUM:
  P1: qkvT[e, tok] = w_qkvT^T @ xT           (xT pre-transposed on host)
  P2: ST[k, q] = kT^T @ qT  -> exp -> PV and row-sum both as matmuls
      (softmax normalization via reciprocal + partition_broadcast)
  P3: out_partial[tok, e] = outT^T @ w_oT    (outT kept SBUF-resident)

RoPE is applied on interleaved even/odd pairs via a DVE stream_shuffle
pair swap and a sign-folded sin table.
"""

import math
import sys
from contextlib import ExitStack

import numpy as np

sys.path.insert(0, "/opt/trn_rl_repo")

import ml_dtypes

BF16NP = ml_dtypes.bfloat16

import concourse.bass as bass
import concourse.mybir as mybir
import concourse.tile as tile
from concourse import bacc

F32 = mybir.dt.float32
F32R = mybir.dt.float32r
BF16 = mybir.dt.bfloat16

B, T, D = 2, 2048, 4096
H, G, HD = 32, 8, 128
THETA = 10000.0
NCORES = 8
HL = H // NCORES            # 4 local q heads
TOK = B * T                 # 4096
QROWS = HL * HD             # 512 local q rows
E = QROWS + 2 * HD          # 768 local qkv rows
SCALE = 1.0 / math.sqrt(HD)

TOKG = 256                  # P1 token-group width
NTOKG = TOK // TOKG
NDC = D // 128              # 32 contraction chunks
NE = E // 128               # 6 qkv row chunks
QG = 512                    # P2 query-group width (within batch)
NQG = T // QG               # 4
NKC = T // 128              # 16 key chunks per batch
MASK_NEG = -1.0e30


def _mask_plan(window: int):
    """Per (qgroup, kchunk): 'skip', 'full', or a mask-key (delta-based)."""
    plan = {}
    keys = {}
    for g in range(NQG):
        for kc in range(NKC):
            i_min, i_max = QG * g, QG * g + QG - 1
            j_min, j_max = 128 * kc, 128 * kc + 127
            if j_min > i_max or (i_min - j_max) >= window:
                plan[(g, kc)] = ("skip", None)
            elif j_max <= i_min and (i_max - j_min) < window:
                plan[(g, kc)] = ("full", None)
            else:
                key = QG * g - 128 * kc
                if key not in keys:
                    keys[key] = len(keys)
                plan[(g, kc)] = ("mask", keys[key])
    return plan, keys


def _build_masks(window: int, keys: dict) -> np.ndarray:
    n = max(1, len(keys))
    m = np.zeros((n, 128, QG), dtype=np.float32)  # cast to bf16 in kernel()
    for key, idx in keys.items():
        # i = key + 128*kc ... i - j = key + qq - kk
        qq = np.arange(QG)[None, :]
        kk = np.arange(128)[:, None]
        diff = key + qq - kk          # i - j
        vis = (diff >= 0) & (diff < window)
        m[idx] = np.where(vis, 1.0, 0.0)
    return m


PAIRSWAP = [i ^ 1 for i in range(32)]


def _rope_ops(nc, pool, dst, src, cos_ap, sin_ap):
    """Interleaved-pair RoPE: dst = src*cos + pairswap(src)*signed_sin.

    cos_ap rows (2i, 2i+1) hold cos_i; sin_ap rows hold (-sin_i, +sin_i).
    src may alias dst (in-place).
    """
    W = dst.shape[-1]
    sw = pool.tile([128, W], BF16, tag="rope_sw")
    tmp = pool.tile([128, W], BF16, tag="rope_tmp")
    qc = pool.tile([128, W], BF16, tag="rope_qc")
    mult = mybir.AluOpType.mult
    nc.vector.stream_shuffle(sw, src, PAIRSWAP)
    nc.vector.tensor_tensor(tmp, sw, sin_ap, mult)
    nc.vector.tensor_tensor(qc, src, cos_ap, mult)
    nc.vector.tensor_tensor(dst, qc, tmp, mybir.AluOpType.add)


class _PhaseStop(Exception):
    pass


def build_nc(window: int, phases=(1, 2, 3)):
    plan, keys = _mask_plan(window)
    nmask = max(1, len(keys))

    nc = bacc.Bacc()
    xT_d = nc.dram_tensor("xT", [D, TOK], BF16, kind="ExternalInput")
    wqkvT_d = nc.dram_tensor("wqkvT", [D, E], BF16, kind="ExternalInput")
    woT_d = nc.dram_tensor("woT", [QROWS, D], BF16, kind="ExternalInput")
    cos_d = nc.dram_tensor("cosH", [128, T], BF16, kind="ExternalInput")
    sin_d = nc.dram_tensor("sinH", [128, T], BF16, kind="ExternalInput")
    masks_d = nc.dram_tensor("masks", [nmask, 128, QG], BF16, kind="ExternalInput")
    ident_d = nc.dram_tensor("ident", [128, 128], BF16, kind="ExternalInput")
    out_d = nc.dram_tensor("out", [TOK, D], F32, kind="ExternalOutput")

    with ExitStack() as octx:
        tc = octx.enter_context(tile.TileContext(nc))
        qkvp = octx.enter_context(tc.tile_pool(name="qkvT", bufs=1))
        qkvT_sb = [qkvp.tile([128, TOK], BF16, tag=f"qkv{e}", name=f"qkv{e}")
                   for e in range(NE)]

        # ---------------- P1: qkvT = w^T @ xT ----------------
        if 1 in phases:
         with ExitStack() as ctx:
            wpool = ctx.enter_context(tc.tile_pool(name="w1", bufs=1))
            xpool = ctx.enter_context(tc.tile_pool(name="x1", bufs=3))
            ppool = ctx.enter_context(tc.tile_pool(name="ps1", bufs=6, space="PSUM"))

            wsb = wpool.tile([128, NDC, E], BF16)
            wq_r = wqkvT_d[:].rearrange("(dc p) e -> p dc e", p=128)
            for dc in range(NDC):
                nc.sync.dma_start(out=wsb[:, dc, :], in_=wq_r[:, dc, :])
            for g in range(NTOKG):
                xsb = xpool.tile([128, NDC, TOKG], BF16, tag="xslab")
                x_r = xT_d[:, g * TOKG:(g + 1) * TOKG].rearrange(
                    "(dc p) t -> p dc t", p=128)
                for dq in range(4):
                    nc.sync.dma_start(out=xsb[:, dq * 8:(dq + 1) * 8, :],
                                      in_=x_r[:, dq * 8:(dq + 1) * 8, :])
                for e in range(NE):
                    ps = ppool.tile([128, TOKG], F32, tag="p1")
                    for dc in range(NDC):
                        nc.tensor.matmul(
                            ps,
                            lhsT=wsb[:, dc, e * 128:(e + 1) * 128],
                            rhs=xsb[:, dc, :],
                            start=(dc == 0), stop=(dc == NDC - 1))
                    # fold softmax 1/sqrt(HD) into q rows; evict into the
                    # SBUF-resident qkvT directly
                    nc.scalar.mul(
                        qkvT_sb[e][:, g * TOKG:(g + 1) * TOKG], ps,
                        SCALE if e < HL else 1.0)

        # ---------------- P2: attention ----------------
        if 2 in phases:
            # outT survives P2 -> P3: allocate after P1's pools are released.
            opool = octx.enter_context(tc.tile_pool(name="outT", bufs=1))
            outT = [opool.tile([128, TOK], BF16, tag=f"outT{i}", name=f"outT{i}")
                    for i in range(HL)]
            p2ctx = ExitStack()
            kpool = p2ctx.enter_context(tc.tile_pool(name="kv", bufs=1))
            ksb = qkvT_sb[HL]
            vsb = kpool.tile([128, TOK // 128, 128], BF16, tag="v")
            cos_sb = kpool.tile([128, T], BF16, tag="cos")
            sin_sb = kpool.tile([128, T], BF16, tag="sin")
            ones_sb = kpool.tile([128, 1], BF16, tag="ones")
            mask_sb = kpool.tile([128, nmask, QG], BF16, tag="masks")

            nc.sync.dma_start(out=cos_sb, in_=cos_d[:])
            nc.sync.dma_start(out=sin_sb, in_=sin_d[:])
            nc.sync.dma_start(
                out=mask_sb, in_=masks_d[:].rearrange("n p q -> p n q"))
            nc.vector.memset(ones_sb, 1.0)

            with ExitStack() as ctx:
                sc0 = ctx.enter_context(tc.tile_pool(name="p2a", bufs=1))
                pt0 = ctx.enter_context(tc.tile_pool(name="p2aps", bufs=2, space="PSUM"))
                ident = sc0.tile([128, 128], BF16, tag="ident")
                nc.sync.dma_start(out=ident, in_=ident_d[:])
                vT = qkvT_sb[HL + 1]
                for tc32 in range(TOK // 128):
                    pst = pt0.tile([128, 128], BF16, tag="tr")
                    nc.tensor.transpose(
                        pst, vT[:, tc32 * 128:(tc32 + 1) * 128], ident)
                    nc.scalar.copy(vsb[:, tc32, :], pst)
                # RoPE on k (per batch)
                for b in range(B):
                    kslice = ksb[:, b * T:(b + 1) * T]
                    _rope_ops(nc, sc0, kslice, kslice, cos_sb, sin_sb)

            # P2 attention interleaved with P3 (output projection): after the 4
            # head-instances of a (batch, q-group) window finish, that window's
            # outT columns are final, so its P3 tiles are emitted immediately —
            # the Tile scheduler uses them to fill PE gaps in later P2 windows.
            with ExitStack() as ctx:
                qpool = ctx.enter_context(tc.tile_pool(name="q2", bufs=4))
                spool = ctx.enter_context(tc.tile_pool(name="sc2", bufs=4))
                estp = ctx.enter_context(tc.tile_pool(name="est", bufs=6))
                wpool = ctx.enter_context(tc.tile_pool(name="wo", bufs=1))
                panp = ctx.enter_context(tc.tile_pool(name="pan", bufs=2))
                stps = ctx.enter_context(tc.tile_pool(name="stps", bufs=3, space="PSUM"))
                rps = ctx.enter_context(tc.tile_pool(name="rps", bufs=1, space="PSUM"))
                ops = ctx.enter_context(tc.tile_pool(name="ops", bufs=2, space="PSUM"))
                pps = ctx.enter_context(tc.tile_pool(name="ps3", bufs=2, space="PSUM"))

                wo = []
                for dc in range(HL):
                    w = wpool.tile([128, D], BF16, tag=f"wo{dc}", name=f"wo{dc}")
                    nc.sync.dma_start(
                        out=w, in_=woT_d[dc * 128:(dc + 1) * 128, :])
                    wo.append(w)

                for b in range(B):
                    for g in range(NQG):
                        for hh in range(HL):
                            qsb = qpool.tile([128, QG], BF16, tag="q")
                            _rope_ops(nc, qpool, qsb,
                                      qkvT_sb[hh][:, b * T + g * QG:
                                                  b * T + (g + 1) * QG],
                                      cos_sb[:, g * QG:(g + 1) * QG],
                                      sin_sb[:, g * QG:(g + 1) * QG])
                            vis = [(kc, plan[(g, kc)]) for kc in range(NKC)
                                   if plan[(g, kc)][0] != "skip"]
                            r_ps = rps.tile([1, QG], F32, tag="r")
                            o_ps = ops.tile([128, QG], F32, tag="o")
                            for idx, (kc, (kind, mid)) in enumerate(vis):
                                # visible query subrange of this key chunk:
                                # qq >= -aoff (causal), qq < w - aoff + 127
                                aoff = QG * g - 128 * kc
                                qlo = max(0, -aoff)
                                qhi = min(QG, window - aoff + 127)
                                qsl = slice(qlo, qhi)
                                st = stps.tile([128, QG], F32, tag="st")
                                nc.tensor.matmul(
                                    st[:, qsl],
                                    lhsT=ksb[:, b * T + kc * 128:
                                             b * T + (kc + 1) * 128],
                                    rhs=qsb[:, qsl],
                                    start=True, stop=True)
                                est = estp.tile([128, QG], BF16, tag="est")
                                nc.scalar.activation(
                                    est[:, qsl], st[:, qsl],
                                    mybir.ActivationFunctionType.Exp)
                                if kind == "mask":
                                    nc.vector.tensor_tensor(
                                        est[:, qsl], est[:, qsl],
                                        mask_sb[:, mid, qsl],
                                        mybir.AluOpType.mult)
                                last = idx == len(vis) - 1
                                nc.tensor.matmul(
                                    r_ps[:, qsl], lhsT=ones_sb,
                                    rhs=est[:, qsl],
                                    start=(idx == 0), stop=last)
                                nc.tensor.matmul(
                                    o_ps[:, qsl],
                                    lhsT=vsb[:, b * NKC + kc, :],
                                    rhs=est[:, qsl],
                                    start=(idx == 0), stop=last)
                            rrec = spool.tile([1, QG], F32, tag="rrec")
                            nc.vector.reciprocal(rrec, r_ps)
                            rb = spool.tile([128, QG], F32, tag="rb")
                            nc.gpsimd.partition_broadcast(rb, rrec)
                            nc.vector.tensor_tensor(
                                outT[hh][:, b * T + g * QG: b * T + (g + 1) * QG],
                                o_ps, rb, mybir.AluOpType.mult)

                        # P3 for this window's 4 token chunks
                        for tloc in range(QG // 128):
                            tch = (b * T + g * QG) // 128 + tloc
                            panel = panp.tile([128, D], F32, tag="panel")
                            for et in range(D // 512):
                                ps = pps.tile([128, 512], F32, tag="p3")
                                for dc in range(HL):
                                    nc.tensor.matmul(
                                        ps,
                                        lhsT=outT[dc][:,
                                                      tch * 128:(tch + 1) * 128],
                                        rhs=wo[dc][:, et * 512:(et + 1) * 512],
                                        start=(dc == 0), stop=(dc == HL - 1))
                                nc.scalar.copy(
                                    panel[:, et * 512:(et + 1) * 512], ps)
                            nc.sync.dma_start(
                                out=out_d[tch * 128:(tch + 1) * 128, :], in_=panel)

            p2ctx.close()

    nc.finalize()
    return nc, nmask


_CACHE = {}


def _get_nc(window: int):
    if window not in _CACHE:
        _CACHE[window] = build_nc(window)
    return _CACHE[window]


LAST_RESULTS = None


def kernel(x, w_qkv, w_o, window_size, _trace=False):
    window = int(window_size)
    nc, nmask = _get_nc(window)
    _, keys = _mask_plan(window)
    masks = _build_masks(window, keys)

    xT = np.ascontiguousarray(x.reshape(TOK, D).T).astype(BF16NP)

    inv = 1.0 / (THETA ** (np.arange(0, HD, 2, dtype=np.float64) / HD))
    freqs = np.arange(T, dtype=np.float64)[:, None] * inv[None, :]  # [T, 64]
    cosH = np.repeat(np.cos(freqs).T, 2, axis=0).astype(BF16NP)  # [128, T]
    sign = np.where(np.arange(HD) % 2 == 0, -1.0, 1.0)[:, None]
    sinH = (np.repeat(np.sin(freqs).T, 2, axis=0) * sign).astype(BF16NP)
    ident = np.eye(128).astype(BF16NP)

    in_maps = []
    for c in range(NCORES):
        wq = w_qkv[QROWS * c:QROWS * (c + 1)]
        wk = w_qkv[H * HD + HD * c: H * HD + HD * (c + 1)]
        wv = w_qkv[H * HD + G * HD + HD * c: H * HD + G * HD + HD * (c + 1)]
        wqkvT = np.ascontiguousarray(
            np.concatenate([wq, wk, wv], axis=0).T).astype(BF16NP)
        woT = np.ascontiguousarray(
            w_o[:, QROWS * c:QROWS * (c + 1)].T).astype(BF16NP)
        in_maps.append({
            "xT": xT, "wqkvT": wqkvT, "woT": woT,
            "cosH": cosH, "sinH": sinH, "masks": masks.astype(BF16NP),
            "ident": ident,
        })

    from concourse.bass_utils import run_bass_kernel_spmd
    res = run_bass_kernel_spmd(nc, in_maps, core_ids=list(range(NCORES)),
                               trace=_trace)
    global LAST_RESULTS
    LAST_RESULTS = res
    acc = res.results[0]["out"].astype(np.float32).copy()
    for c in range(1, NCORES):
        acc += res.results[c]["out"]
    return acc.reshape(B, T, D)



# revision 3
# speedup vs baseline: 1.1754x; 1.1754x over previous
"""Trainium2 Bass kernel: fused QKV + RoPE + causal/windowed GQA attention + output proj.

Sharding: tensor-parallel by head across 8 cores. Core c owns Q-heads
4c..4c+3 and KV-group c (matching repeat_interleave grouping), plus the
512 w_o columns for those heads. Each core computes a full-shape partial
of the final output (contraction over its 512 attention-output dims);
the host sums the 8 partials. No device collectives.

The two big GEMMs (QKV projection P1 and output projection P3) run as
fp8e4 matmuls in DoubleRow perf mode (2 contraction k-tiles per
instruction at 0.5 cycles/row = 4x bf16 FLOP rate). Full bf16-grade
accuracy is kept with a 3-product hi/lo split per operand:
    A@B ~= Ah@Bh + Ah@Bl + Al@Bh          (Al,Bl = e4m3 residuals)
The lo planes are stored UNSCALED (e4m3 subnormals give a 2^-10 fixed
point grid there), so all 3 products share one scale and accumulate in
a single PSUM group. Operands whose scale is small (weights, sigma
1/64) are pre-scaled by 64 on the host; the 64x factors ride through
the pipeline (qkv is stored as 64x, the exp activation's scale arg
divides them back out, attention output is stored as 32x, and the host
divides the final partials by 2048).

P2 (attention) stays bf16: ST[k,q] = kT^T @ qT -> exp -> PV and row-sum
both as matmuls (softmax normalization via reciprocal + broadcast).
RoPE is applied on interleaved even/odd pairs via a DVE stream_shuffle
pair swap and a sign-folded sin table.
"""

import math
import sys
from contextlib import ExitStack

import numpy as np

sys.path.insert(0, "/opt/trn_rl_repo")

import ml_dtypes

BF16NP = ml_dtypes.bfloat16
F8NP = ml_dtypes.float8_e4m3

import concourse.bass as bass
import concourse.mybir as mybir
import concourse.tile as tile
from concourse import bacc

F32 = mybir.dt.float32
BF16 = mybir.dt.bfloat16
FP8 = mybir.dt.float8e4
DR = mybir.MatmulPerfMode.DoubleRow

B, T, D = 2, 2048, 4096
H, G, HD = 32, 8, 128
THETA = 10000.0
NCORES = 8
HL = H // NCORES            # 4 local q heads
TOK = B * T                 # 4096
QROWS = HL * HD             # 512 local q rows
E = QROWS + 2 * HD          # 768 local qkv rows
SCALE = 1.0 / math.sqrt(HD)
ESCALE = SCALE / 4096.0     # exp scale: q,k each carry a 64x factor

TOKG = 256                  # P1 token-group width
NTOKG = TOK // TOKG
NDC = D // 128              # 32 contraction chunks
NE = E // 128               # 6 qkv row chunks
QG = 512                    # P2 query-group width (within batch)
NQG = T // QG               # 4
NKC = T // 128              # 16 key chunks per batch


def _mask_plan(window: int):
    """Per (qgroup, kchunk): 'skip', 'full', or a mask-key (delta-based)."""
    plan = {}
    keys = {}
    for g in range(NQG):
        for kc in range(NKC):
            i_min, i_max = QG * g, QG * g + QG - 1
            j_min, j_max = 128 * kc, 128 * kc + 127
            if j_min > i_max or (i_min - j_max) >= window:
                plan[(g, kc)] = ("skip", None)
            elif j_max <= i_min and (i_max - j_min) < window:
                plan[(g, kc)] = ("full", None)
            else:
                key = QG * g - 128 * kc
                if key not in keys:
                    keys[key] = len(keys)
                plan[(g, kc)] = ("mask", keys[key])
    return plan, keys


def _build_masks(window: int, keys: dict) -> np.ndarray:
    n = max(1, len(keys))
    m = np.zeros((n, 128, QG), dtype=np.float32)  # cast to bf16 in kernel()
    for key, idx in keys.items():
        # i = key + 128*kc ... i - j = key + qq - kk
        qq = np.arange(QG)[None, :]
        kk = np.arange(128)[:, None]
        diff = key + qq - kk          # i - j
        vis = (diff >= 0) & (diff < window)
        m[idx] = np.where(vis, 1.0, 0.0)
    return m


PAIRSWAP = [i ^ 1 for i in range(32)]


def _rope_ops(nc, pool, dst, src, cos_ap, sin_ap):
    """Interleaved-pair RoPE: dst = src*cos + pairswap(src)*signed_sin.

    cos_ap rows (2i, 2i+1) hold cos_i; sin_ap rows hold (-sin_i, +sin_i).
    src may alias dst (in-place).
    """
    W = dst.shape[-1]
    sw = pool.tile([128, W], BF16, tag="rope_sw")
    tmp = pool.tile([128, W], BF16, tag="rope_tmp")
    qc = pool.tile([128, W], BF16, tag="rope_qc")
    mult = mybir.AluOpType.mult
    nc.vector.stream_shuffle(sw, src, PAIRSWAP)
    nc.vector.tensor_tensor(tmp, sw, sin_ap, mult)
    nc.vector.tensor_tensor(qc, src, cos_ap, mult)
    nc.vector.tensor_tensor(dst, qc, tmp, mybir.AluOpType.add)


def build_nc(window: int):
    plan, keys = _mask_plan(window)
    nmask = max(1, len(keys))

    nc = bacc.Bacc()
    x8_d = nc.dram_tensor("x8", [128, NDC, 2, TOK], FP8, kind="ExternalInput")
    w8_d = nc.dram_tensor("w8", [128, NDC, 2, E], FP8, kind="ExternalInput")
    wo8_d = nc.dram_tensor("wo8", [128, HL, 2, D], FP8, kind="ExternalInput")
    cos_d = nc.dram_tensor("cosH", [128, T], BF16, kind="ExternalInput")
    sin_d = nc.dram_tensor("sinH", [128, T], BF16, kind="ExternalInput")
    masks_d = nc.dram_tensor("masks", [nmask, 128, QG], BF16, kind="ExternalInput")
    ident_d = nc.dram_tensor("ident", [128, 128], BF16, kind="ExternalInput")
    out_d = nc.dram_tensor("out", [TOK, D], BF16, kind="ExternalOutput")

    with ExitStack() as octx:
        tc = octx.enter_context(tile.TileContext(nc))
        qkvp = octx.enter_context(tc.tile_pool(name="qkvT", bufs=1))
        qkvT_sb = [qkvp.tile([128, TOK], BF16, tag=f"qkv{e}", name=f"qkv{e}")
                   for e in range(NE)]

        # ---------------- P1: qkvT(64x) = (64 w)^T @ x, fp8 DoubleRow ---------
        with ExitStack() as ctx:
            wpool = ctx.enter_context(tc.tile_pool(name="w1", bufs=1))
            xpool = ctx.enter_context(tc.tile_pool(name="x1", bufs=3))
            ppool = ctx.enter_context(tc.tile_pool(name="ps1", bufs=6, space="PSUM"))

            wsb = wpool.tile([128, NDC, 2, E], FP8)
            for dc in range(NDC):
                nc.sync.dma_start(out=wsb[:, dc], in_=w8_d[:, dc])
            for g in range(NTOKG):
                xsb = xpool.tile([128, NDC, 2, TOKG], FP8, tag="xslab")
                for dq in range(4):
                    nc.sync.dma_start(
                        out=xsb[:, dq * 8:(dq + 1) * 8],
                        in_=x8_d[:, dq * 8:(dq + 1) * 8, :,
                                 g * TOKG:(g + 1) * TOKG])
                for e in range(NE):
                    ps = ppool.tile([128, TOKG], F32, tag="p1")
                    es = slice(e * 128, (e + 1) * 128)
                    # main: (w_hi, x_hi) over chunk pairs
                    for p in range(NDC // 2):
                        nc.tensor.matmul(
                            ps,
                            lhsT=wsb[:, 2 * p:2 * p + 2, 0, es],
                            rhs=xsb[:, 2 * p:2 * p + 2, 1, :],
                            start=(p == 0), stop=False, perf_mode=DR)
                    # corr: (w_hi x_lo) + (w_lo x_hi) per chunk
                    for dc in range(NDC):
                        nc.tensor.matmul(
                            ps,
                            lhsT=wsb[:, dc, 0:2, es],
                            rhs=xsb[:, dc, 0:2, :],
                            start=False, stop=(dc == NDC - 1), perf_mode=DR)
                    nc.scalar.copy(
                        qkvT_sb[e][:, g * TOKG:(g + 1) * TOKG], ps)

        # ---------------- P2: attention ----------------
        # oT8 survives P2 -> P3: hi/lo fp8 planes of 32*(o/Z).
        opool = octx.enter_context(tc.tile_pool(name="outT", bufs=1))
        oT8 = opool.tile([128, HL, 2, TOK], FP8, name="oT8")
        p2ctx = ExitStack()
        kpool = p2ctx.enter_context(tc.tile_pool(name="kv", bufs=1))
        ksb = qkvT_sb[HL]
        vsb = kpool.tile([128, TOK // 128, 128], BF16, tag="v")
        cos_sb = kpool.tile([128, T], BF16, tag="cos")
        sin_sb = kpool.tile([128, T], BF16, tag="sin")
        ones_sb = kpool.tile([128, 1], BF16, tag="ones")
        mask_sb = kpool.tile([128, nmask, QG], BF16, tag="masks")

        nc.sync.dma_start(out=cos_sb, in_=cos_d[:])
        nc.sync.dma_start(out=sin_sb, in_=sin_d[:])
        nc.sync.dma_start(
            out=mask_sb, in_=masks_d[:].rearrange("n p q -> p n q"))
        # rowsum weights of 2.0 fold the 1/2 of the 32x output scale into
        # the reciprocal: rrec = 1/(2Z)
        nc.vector.memset(ones_sb, 2.0)

        with ExitStack() as ctx:
            sc0 = ctx.enter_context(tc.tile_pool(name="p2a", bufs=1))
            pt0 = ctx.enter_context(tc.tile_pool(name="p2aps", bufs=2, space="PSUM"))
            ident = sc0.tile([128, 128], BF16, tag="ident")
            nc.sync.dma_start(out=ident, in_=ident_d[:])
            vT = qkvT_sb[HL + 1]
            for tc32 in range(TOK // 128):
                pst = pt0.tile([128, 128], BF16, tag="tr")
                nc.tensor.transpose(
                    pst, vT[:, tc32 * 128:(tc32 + 1) * 128], ident)
                nc.scalar.copy(vsb[:, tc32, :], pst)
            # RoPE on k (per batch)
            for b in range(B):
                kslice = ksb[:, b * T:(b + 1) * T]
                _rope_ops(nc, sc0, kslice, kslice, cos_sb, sin_sb)

        # P2 attention interleaved with P3 (output projection): after the 4
        # head-instances of a (batch, q-group) window finish, that window's
        # oT8 columns are final, so its P3 tiles are emitted immediately -
        # the Tile scheduler uses them to fill PE gaps in later P2 windows.
        with ExitStack() as ctx:
            qpool = ctx.enter_context(tc.tile_pool(name="q2", bufs=4))
            spool = ctx.enter_context(tc.tile_pool(name="sc2", bufs=4))
            estp = ctx.enter_context(tc.tile_pool(name="est", bufs=6))
            wpool = ctx.enter_context(tc.tile_pool(name="wo", bufs=1))
            panp = ctx.enter_context(tc.tile_pool(name="pan", bufs=2))
            stps = ctx.enter_context(tc.tile_pool(name="stps", bufs=3, space="PSUM"))
            rps = ctx.enter_context(tc.tile_pool(name="rps", bufs=1, space="PSUM"))
            ops = ctx.enter_context(tc.tile_pool(name="ops", bufs=2, space="PSUM"))
            pps = ctx.enter_context(tc.tile_pool(name="ps3", bufs=2, space="PSUM"))

            wo8 = wpool.tile([128, HL, 2, D], FP8, name="wo8")
            for dc in range(HL):
                nc.sync.dma_start(out=wo8[:, dc], in_=wo8_d[:, dc])

            for b in range(B):
                for g in range(NQG):
                    for hh in range(HL):
                        qsb = qpool.tile([128, QG], BF16, tag="q")
                        _rope_ops(nc, qpool, qsb,
                                  qkvT_sb[hh][:, b * T + g * QG:
                                              b * T + (g + 1) * QG],
                                  cos_sb[:, g * QG:(g + 1) * QG],
                                  sin_sb[:, g * QG:(g + 1) * QG])
                        vis = [(kc, plan[(g, kc)]) for kc in range(NKC)
                               if plan[(g, kc)][0] != "skip"]
                        r_ps = rps.tile([1, QG], F32, tag="r")
                        o_ps = ops.tile([128, QG], F32, tag="o")
                        for idx, (kc, (kind, mid)) in enumerate(vis):
                            # visible query subrange of this key chunk:
                            # qq >= -aoff (causal), qq < w - aoff + 127
                            aoff = QG * g - 128 * kc
                            qlo = max(0, -aoff)
                            qhi = min(QG, window - aoff + 127)
                            qsl = slice(qlo, qhi)
                            st = stps.tile([128, QG], F32, tag="st")
                            nc.tensor.matmul(
                                st[:, qsl],
                                lhsT=ksb[:, b * T + kc * 128:
                                         b * T + (kc + 1) * 128],
                                rhs=qsb[:, qsl],
                                start=True, stop=True)
                            est = estp.tile([128, QG], BF16, tag="est")
                            nc.scalar.activation(
                                est[:, qsl], st[:, qsl],
                                mybir.ActivationFunctionType.Exp,
                                scale=ESCALE)
                            if kind == "mask":
                                nc.vector.tensor_tensor(
                                    est[:, qsl], est[:, qsl],
                                    mask_sb[:, mid, qsl],
                                    mybir.AluOpType.mult)
                            last = idx == len(vis) - 1
                            nc.tensor.matmul(
                                r_ps[:, qsl], lhsT=ones_sb,
                                rhs=est[:, qsl],
                                start=(idx == 0), stop=last)
                            nc.tensor.matmul(
                                o_ps[:, qsl],
                                lhsT=vsb[:, b * NKC + kc, :],
                                rhs=est[:, qsl],
                                start=(idx == 0), stop=last)
                        rrec = spool.tile([1, QG], F32, tag="rrec")
                        nc.vector.reciprocal(rrec, r_ps)
                        rb = spool.tile([128, QG], F32, tag="rb")
                        nc.gpsimd.partition_broadcast(rb, rrec)
                        # tb = o_ps/(2Z) = 32*(o/Z); split into fp8 hi+lo
                        wsl = slice(b * T + g * QG, b * T + (g + 1) * QG)
                        tb = spool.tile([128, QG], BF16, tag="tb")
                        nc.vector.tensor_tensor(
                            tb, o_ps, rb, mybir.AluOpType.mult)
                        nc.scalar.copy(oT8[:, hh, 0, wsl], tb)
                        nc.vector.scalar_tensor_tensor(
                            oT8[:, hh, 1, wsl], oT8[:, hh, 0, wsl], -1.0, tb,
                            mybir.AluOpType.mult, mybir.AluOpType.add)

                    # P3 for this window's 4 token chunks, fp8 DoubleRow
                    for tloc in range(QG // 128):
                        tch = (b * T + g * QG) // 128 + tloc
                        tsl = slice(tch * 128, (tch + 1) * 128)
                        panel = panp.tile([128, D], BF16, tag="panel")
                        for et in range(D // 512):
                            ps = pps.tile([128, 512], F32, tag="p3")
                            esl = slice(et * 512, (et + 1) * 512)
                            for p in range(HL // 2):
                                nc.tensor.matmul(
                                    ps,
                                    lhsT=oT8[:, 2 * p:2 * p + 2, 0, tsl],
                                    rhs=wo8[:, 2 * p:2 * p + 2, 1, esl],
                                    start=(p == 0), stop=False, perf_mode=DR)
                            for dc in range(HL):
                                nc.tensor.matmul(
                                    ps,
                                    lhsT=oT8[:, dc, 0:2, tsl],
                                    rhs=wo8[:, dc, 0:2, esl],
                                    start=False, stop=(dc == HL - 1),
                                    perf_mode=DR)
                            nc.scalar.copy(panel[:, esl], ps)
                        nc.sync.dma_start(out=out_d[tsl, :], in_=panel)

        p2ctx.close()

    nc.finalize()
    return nc, nmask


_CACHE = {}


def _get_nc(window: int):
    if window not in _CACHE:
        _CACHE[window] = build_nc(window)
    return _CACHE[window]


def _split8(a):
    """e4m3 hi + unscaled lo residual planes of a [R, C] f32 array,
    interleaved to [128, R//128, 2, C] with (plane0, plane1) = order."""
    hi = a.astype(F8NP)
    lo = (a - hi.astype(np.float32)).astype(F8NP)
    return hi, lo


def _plane_pack(hi, lo, first, second):
    """Pack [R, C] planes into [128, R//128, 2, C] (p, chunk, plane, col)."""
    R, C = hi.shape
    out = np.empty((128, R // 128, 2, C), dtype=F8NP)
    sel = {"hi": hi, "lo": lo}
    out[:, :, 0, :] = sel[first].reshape(R // 128, 128, C).transpose(1, 0, 2)
    out[:, :, 1, :] = sel[second].reshape(R // 128, 128, C).transpose(1, 0, 2)
    return out


LAST_RESULTS = None


def kernel(x, w_qkv, w_o, window_size, _trace=False):
    window = int(window_size)
    nc, nmask = _get_nc(window)
    _, keys = _mask_plan(window)
    masks = _build_masks(window, keys)

    # x: stationary-side convention is (hi, lo); moving side is (lo, hi).
    xT = np.ascontiguousarray(
        x.reshape(TOK, D).T).astype(np.float32)          # [D, TOK]
    xh, xl = _split8(xT)
    x8 = _plane_pack(xh, xl, "lo", "hi")                 # moving

    inv = 1.0 / (THETA ** (np.arange(0, HD, 2, dtype=np.float64) / HD))
    freqs = np.arange(T, dtype=np.float64)[:, None] * inv[None, :]  # [T, 64]
    cosH = np.repeat(np.cos(freqs).T, 2, axis=0).astype(BF16NP)  # [128, T]
    sign = np.where(np.arange(HD) % 2 == 0, -1.0, 1.0)[:, None]
    sinH = (np.repeat(np.sin(freqs).T, 2, axis=0) * sign).astype(BF16NP)
    ident = np.eye(128).astype(BF16NP)

    in_maps = []
    for c in range(NCORES):
        wq = w_qkv[QROWS * c:QROWS * (c + 1)]
        wk = w_qkv[H * HD + HD * c: H * HD + HD * (c + 1)]
        wv = w_qkv[H * HD + G * HD + HD * c: H * HD + G * HD + HD * (c + 1)]
        W = np.ascontiguousarray(
            np.concatenate([wq, wk, wv], axis=0).T).astype(np.float32) * 64.0
        wh, wl = _split8(W)                              # [D, E]
        w8 = _plane_pack(wh, wl, "hi", "lo")             # stationary
        WO = np.ascontiguousarray(
            w_o[:, QROWS * c:QROWS * (c + 1)].T).astype(np.float32) * 64.0
        woh, wol = _split8(WO)                           # [QROWS, D]
        wo8 = _plane_pack(woh, wol, "lo", "hi")          # moving
        in_maps.append({
            "x8": x8, "w8": w8, "wo8": wo8,
            "cosH": cosH, "sinH": sinH, "masks": masks.astype(BF16NP),
            "ident": ident,
        })

    from concourse.bass_utils import run_bass_kernel_spmd
    res = run_bass_kernel_spmd(nc, in_maps, core_ids=list(range(NCORES)),
                               trace=_trace)
    global LAST_RESULTS
    LAST_RESULTS = res
    acc = res.results[0]["out"].astype(np.float32)
    for c in range(1, NCORES):
        acc = acc + res.results[c]["out"].astype(np.float32)
    # undo the 32x (oT8) * 64x (wo8) operand scaling
    return (acc / 2048.0).reshape(B, T, D)


# revision 7
# speedup vs baseline: 1.2101x; 1.0295x over previous
"""Trainium2 Bass kernel: fused QKV + RoPE + causal/windowed GQA attention + output proj.

Sharding: tensor-parallel by head across 8 cores. Core c owns Q-heads
4c..4c+3 and KV-group c (matching repeat_interleave grouping), plus the
512 w_o columns for those heads. Each core computes a full-shape partial
of the final output (contraction over its 512 attention-output dims);
the host sums the 8 partials. No device collectives.

The two big GEMMs (QKV projection P1 and output projection P3) run as
fp8e4 matmuls in DoubleRow perf mode (2 contraction k-tiles per
instruction at 0.5 cycles/row = 4x bf16 FLOP rate). Full bf16-grade
accuracy is kept with a 3-product hi/lo split per operand:
    A@B ~= Ah@Bh + Ah@Bl + Al@Bh          (Al,Bl = e4m3 residuals)
The lo planes are stored UNSCALED (e4m3 subnormals give a 2^-10 fixed
point grid there), so all 3 products share one scale and accumulate in
a single PSUM group. Operands whose scale is small (weights, sigma
1/64) are pre-scaled by 64 on the host; the 64x factors ride through
the pipeline (qkv is stored as 64x, the exp activation's scale arg
divides them back out, attention output is stored as 32x, and the host
divides the final bf16 partials by 2048).

P2 (attention) stays bf16: ST[k, q] = kT^T @ qT -> exp -> PV and
row-sum both as matmuls (softmax normalization via reciprocal +
partition_broadcast). RoPE is applied on interleaved even/odd pairs via
a DVE stream_shuffle pair swap and a sign-folded sin table.

Scheduling: P1 matmuls are emitted dc-ordered (all 6 output-row chains
advance together) so compute tracks the weight DMA stream; w8 loads
issue from the Activation HWDGE queue to run parallel with the x8
stream on SP. The batch-0 v-transposes and k-RoPE run in the shadow of
P1's second half. In P2, all 4 heads' q-RoPEs are emitted up front and
each window's output projection is emitted one window late, so the PE
always has independent work at window boundaries.
"""

import math
import sys
from contextlib import ExitStack

import numpy as np

sys.path.insert(0, "/opt/trn_rl_repo")

import ml_dtypes

BF16NP = ml_dtypes.bfloat16
F8NP = ml_dtypes.float8_e4m3

import concourse.bass as bass
import concourse.mybir as mybir
import concourse.tile as tile
from concourse import bacc

F32 = mybir.dt.float32
BF16 = mybir.dt.bfloat16
FP8 = mybir.dt.float8e4
DR = mybir.MatmulPerfMode.DoubleRow

B, T, D = 2, 2048, 4096
H, G, HD = 32, 8, 128
THETA = 10000.0
NCORES = 8
HL = H // NCORES            # 4 local q heads
TOK = B * T                 # 4096
QROWS = HL * HD             # 512 local q rows
E = QROWS + 2 * HD          # 768 local qkv rows
SCALE = 1.0 / math.sqrt(HD)
ESCALE = SCALE / 4096.0     # exp scale: q,k each carry a 64x factor

TOKG = 256                  # P1 token-group width
NTOKG = TOK // TOKG
NDC = D // 128              # 32 contraction chunks
NE = E // 128               # 6 qkv row chunks
QG = 512                    # P2 query-group width (within batch)
NQG = T // QG               # 4
NKC = T // 128              # 16 key chunks per batch
GPB = NTOKG // B            # P1 token groups per batch


def _mask_plan(window: int):
    """Per (qgroup, kchunk): 'skip', 'full', or a mask-key (delta-based)."""
    plan = {}
    keys = {}
    for g in range(NQG):
        for kc in range(NKC):
            i_min, i_max = QG * g, QG * g + QG - 1
            j_min, j_max = 128 * kc, 128 * kc + 127
            if j_min > i_max or (i_min - j_max) >= window:
                plan[(g, kc)] = ("skip", None)
            elif j_max <= i_min and (i_max - j_min) < window:
                plan[(g, kc)] = ("full", None)
            else:
                key = QG * g - 128 * kc
                if key not in keys:
                    keys[key] = len(keys)
                plan[(g, kc)] = ("mask", keys[key])
    return plan, keys


def _build_masks(window: int, keys: dict) -> np.ndarray:
    n = max(1, len(keys))
    m = np.zeros((n, 128, QG), dtype=np.float32)  # cast to bf16 in kernel()
    for key, idx in keys.items():
        # i = key + 128*kc ... i - j = key + qq - kk
        qq = np.arange(QG)[None, :]
        kk = np.arange(128)[:, None]
        diff = key + qq - kk          # i - j
        vis = (diff >= 0) & (diff < window)
        m[idx] = np.where(vis, 1.0, 0.0)
    return m


PAIRSWAP = [i ^ 1 for i in range(32)]


def _rope_ops(nc, pool, dst, src, cos_ap, sin_ap):
    """Interleaved-pair RoPE: dst = src*cos + pairswap(src)*signed_sin.

    cos_ap rows (2i, 2i+1) hold cos_i; sin_ap rows hold (-sin_i, +sin_i).
    src may alias dst (in-place).
    """
    W = dst.shape[-1]
    sw = pool.tile([128, W], BF16, tag="rope_sw")
    tmp = pool.tile([128, W], BF16, tag="rope_tmp")
    qc = pool.tile([128, W], BF16, tag="rope_qc")
    mult = mybir.AluOpType.mult
    nc.vector.stream_shuffle(sw, src, PAIRSWAP)
    nc.vector.tensor_tensor(tmp, sw, sin_ap, mult)
    nc.vector.tensor_tensor(qc, src, cos_ap, mult)
    nc.vector.tensor_tensor(dst, qc, tmp, mybir.AluOpType.add)


def build_nc(window: int):
    plan, keys = _mask_plan(window)
    nmask = max(1, len(keys))

    nc = bacc.Bacc()
    x8_d = nc.dram_tensor("x8", [128, NDC, 2, TOK], FP8, kind="ExternalInput")
    w8_d = nc.dram_tensor("w8", [128, NDC, 2, E], FP8, kind="ExternalInput")
    wo8_d = nc.dram_tensor("wo8", [128, HL, 2, D], FP8, kind="ExternalInput")
    cos_d = nc.dram_tensor("cosH", [128, T], BF16, kind="ExternalInput")
    sin_d = nc.dram_tensor("sinH", [128, T], BF16, kind="ExternalInput")
    masks_d = nc.dram_tensor("masks", [nmask, 128, QG], BF16, kind="ExternalInput")
    ident_d = nc.dram_tensor("ident", [128, 128], BF16, kind="ExternalInput")
    out_d = nc.dram_tensor("out", [TOK, D], BF16, kind="ExternalOutput")

    with ExitStack() as octx:
        tc = octx.enter_context(tile.TileContext(nc))
        qkvp = octx.enter_context(tc.tile_pool(name="qkvT", bufs=1))
        qkvT_sb = [qkvp.tile([128, TOK], BF16, tag=f"qkv{e}", name=f"qkv{e}")
                   for e in range(NE)]
        opool = octx.enter_context(tc.tile_pool(name="outT", bufs=1))
        oT8 = opool.tile([128, HL, 2, TOK], FP8, name="oT8")
        kvp = octx.enter_context(tc.tile_pool(name="kv", bufs=1))
        sc0 = octx.enter_context(tc.tile_pool(name="p2a", bufs=1))
        vsb = kvp.tile([128, TOK // 128, 128], BF16, tag="v")
        cos_sb = kvp.tile([128, T], BF16, tag="cos")
        sin_sb = kvp.tile([128, T], BF16, tag="sin")
        ones_sb = kvp.tile([128, 1], BF16, tag="ones")
        mask_sb = kvp.tile([128, nmask, QG], BF16, tag="masks")
        ident = kvp.tile([128, 128], BF16, tag="ident")

        nc.sync.dma_start(out=ident, in_=ident_d[:])
        nc.sync.dma_start(out=cos_sb, in_=cos_d[:])
        nc.sync.dma_start(out=sin_sb, in_=sin_d[:])
        nc.sync.dma_start(
            out=mask_sb, in_=masks_d[:].rearrange("n p q -> p n q"))
        # rowsum weights of 2.0 fold the 1/2 of the 32x output scale into
        # the reciprocal: rrec = 1/(2Z)
        nc.vector.memset(ones_sb, 2.0)

        ksb = qkvT_sb[HL]
        vT = qkvT_sb[HL + 1]

        # ---------------- P1: qkvT(64x) = (64 w)^T @ x, fp8 DoubleRow ---------
        with ExitStack() as ctx:
            wpool = ctx.enter_context(tc.tile_pool(name="w1", bufs=1))
            xpool = ctx.enter_context(tc.tile_pool(name="x1", bufs=2))
            ppool = ctx.enter_context(tc.tile_pool(name="ps1", bufs=1, space="PSUM"))
            pt0 = ctx.enter_context(tc.tile_pool(name="p2aps", bufs=2, space="PSUM"))

            # weights stream on the Activation HWDGE queue, x on SP: the two
            # queues run in parallel and P1's dc-ordered chains track them.
            wsb = wpool.tile([128, NDC, 2, E], FP8)
            for dc in range(NDC):
                nc.scalar.dma_start(out=wsb[:, dc], in_=w8_d[:, dc])
            for g in range(NTOKG):
                xsb = xpool.tile([128, NDC, 2, TOKG], FP8, tag="xslab")
                for dq in range(4):
                    nc.sync.dma_start(
                        out=xsb[:, dq * 8:(dq + 1) * 8],
                        in_=x8_d[:, dq * 8:(dq + 1) * 8, :,
                                 g * TOKG:(g + 1) * TOKG])
                pss = [ppool.tile([128, TOKG], F32, tag=f"p1_{e}",
                                  name=f"p1_{e}") for e in range(NE)]
                for p in range(NDC // 2):
                    for e in range(NE):
                        es = slice(e * 128, (e + 1) * 128)
                        # main: (w_hi, x_hi) over the chunk pair
                        nc.tensor.matmul(
                            pss[e],
                            lhsT=wsb[:, 2 * p:2 * p + 2, 0, es],
                            rhs=xsb[:, 2 * p:2 * p + 2, 1, :],
                            start=(p == 0), stop=False, perf_mode=DR)
                        # corr: (w_hi x_lo) + (w_lo x_hi) per chunk
                        for dc in (2 * p, 2 * p + 1):
                            nc.tensor.matmul(
                                pss[e],
                                lhsT=wsb[:, dc, 0:2, es],
                                rhs=xsb[:, dc, 0:2, :],
                                start=False,
                                stop=(dc == NDC - 1), perf_mode=DR)
                for e in range(NE):
                    nc.scalar.copy(
                        qkvT_sb[e][:, g * TOKG:(g + 1) * TOKG], pss[e])

                # batch b's K/V post-processing in the shadow of P1's tail
                if (g + 1) % GPB == 0:
                    b = (g + 1) // GPB - 1
                    for tc32 in range(b * NKC, (b + 1) * NKC):
                        pst = pt0.tile([128, 128], BF16, tag="tr")
                        nc.tensor.transpose(
                            pst, vT[:, tc32 * 128:(tc32 + 1) * 128], ident)
                        nc.scalar.copy(vsb[:, tc32, :], pst)
                    kslice = ksb[:, b * T:(b + 1) * T]
                    _rope_ops(nc, sc0, kslice, kslice, cos_sb, sin_sb)

        # ---------------- P2 attention + interleaved P3 ----------------
        with ExitStack() as ctx:
            qpool = ctx.enter_context(tc.tile_pool(name="q2", bufs=5))
            spool = ctx.enter_context(tc.tile_pool(name="sc2", bufs=4))
            estp = ctx.enter_context(tc.tile_pool(name="est", bufs=6))
            wpool = ctx.enter_context(tc.tile_pool(name="wo", bufs=1))
            panp = ctx.enter_context(tc.tile_pool(name="pan", bufs=2))
            stps = ctx.enter_context(tc.tile_pool(name="stps", bufs=3, space="PSUM"))
            rps = ctx.enter_context(tc.tile_pool(name="rps", bufs=1, space="PSUM"))
            ops = ctx.enter_context(tc.tile_pool(name="ops", bufs=2, space="PSUM"))
            pps = ctx.enter_context(tc.tile_pool(name="ps3", bufs=2, space="PSUM"))

            wo8 = wpool.tile([128, HL, 2, D], FP8, name="wo8")
            for dc in range(HL):
                nc.sync.dma_start(out=wo8[:, dc], in_=wo8_d[:, dc])

            def emit_p3(bw, gw, tlocs):
                # fp8 DoubleRow output projection for finished window (bw,gw)
                for tloc in tlocs:
                    tch = (bw * T + gw * QG) // 128 + tloc
                    tsl = slice(tch * 128, (tch + 1) * 128)
                    panel = panp.tile([128, D], BF16, tag="panel")
                    for et in range(D // 512):
                        ps = pps.tile([128, 512], F32, tag="p3")
                        esl = slice(et * 512, (et + 1) * 512)
                        for p in range(HL // 2):
                            nc.tensor.matmul(
                                ps,
                                lhsT=oT8[:, 2 * p:2 * p + 2, 0, tsl],
                                rhs=wo8[:, 2 * p:2 * p + 2, 1, esl],
                                start=(p == 0), stop=False, perf_mode=DR)
                        for dc in range(HL):
                            nc.tensor.matmul(
                                ps,
                                lhsT=oT8[:, dc, 0:2, tsl],
                                rhs=wo8[:, dc, 0:2, esl],
                                start=False, stop=(dc == HL - 1),
                                perf_mode=DR)
                        if et % 2 == 0:
                            nc.scalar.copy(panel[:, esl], ps)
                        else:
                            nc.vector.tensor_scalar_mul(panel[:, esl], ps, 1.0)
                    nc.sync.dma_start(out=out_d[tsl, :], in_=panel)

            prev = None
            for b in range(B):
                for g in range(NQG):
                    # q-RoPE for all 4 heads up front: DVE leads the PE
                    qsbs = []
                    for hh in range(HL):
                        qsb = qpool.tile([128, QG], BF16, tag="q")
                        _rope_ops(nc, qpool, qsb,
                                  qkvT_sb[hh][:, b * T + g * QG:
                                              b * T + (g + 1) * QG],
                                  cos_sb[:, g * QG:(g + 1) * QG],
                                  sin_sb[:, g * QG:(g + 1) * QG])
                        qsbs.append(qsb)
                    vis = [(kc, plan[(g, kc)]) for kc in range(NKC)
                           if plan[(g, kc)][0] != "skip"]
                    for hh in range(HL):
                        qsb = qsbs[hh]
                        r_ps = rps.tile([1, QG], F32, tag="r")
                        o_ps = ops.tile([128, QG], F32, tag="o")
                        for idx, (kc, (kind, mid)) in enumerate(vis):
                            # visible query subrange of this key chunk:
                            # qq >= -aoff (causal), qq < w - aoff + 127
                            aoff = QG * g - 128 * kc
                            qlo = max(0, -aoff)
                            qhi = min(QG, window - aoff + 127)
                            qsl = slice(qlo, qhi)
                            st = stps.tile([128, QG], F32, tag="st")
                            nc.tensor.matmul(
                                st[:, qsl],
                                lhsT=ksb[:, b * T + kc * 128:
                                         b * T + (kc + 1) * 128],
                                rhs=qsb[:, qsl],
                                start=True, stop=True)
                            est = estp.tile([128, QG], BF16, tag="est")
                            nc.scalar.activation(
                                est[:, qsl], st[:, qsl],
                                mybir.ActivationFunctionType.Exp,
                                scale=ESCALE)
                            if kind == "mask":
                                nc.vector.tensor_tensor(
                                    est[:, qsl], est[:, qsl],
                                    mask_sb[:, mid, qsl],
                                    mybir.AluOpType.mult)
                            last = idx == len(vis) - 1
                            nc.tensor.matmul(
                                r_ps[:, qsl], lhsT=ones_sb,
                                rhs=est[:, qsl],
                                start=(idx == 0), stop=last)
                            nc.tensor.matmul(
                                o_ps[:, qsl],
                                lhsT=vsb[:, b * NKC + kc, :],
                                rhs=est[:, qsl],
                                start=(idx == 0), stop=last)
                        rrec = spool.tile([1, QG], F32, tag="rrec")
                        nc.vector.reciprocal(rrec, r_ps)
                        rb = spool.tile([128, QG], F32, tag="rb")
                        nc.gpsimd.partition_broadcast(rb, rrec)
                        # tb = o_ps/(2Z) = 32*(o/Z); split into fp8 hi+lo
                        wsl = slice(b * T + g * QG, b * T + (g + 1) * QG)
                        tb = spool.tile([128, QG], BF16, tag="tb")
                        nc.vector.tensor_tensor(
                            tb, o_ps, rb, mybir.AluOpType.mult)
                        nc.scalar.copy(oT8[:, hh, 0, wsl], tb)
                        nc.vector.scalar_tensor_tensor(
                            oT8[:, hh, 1, wsl], oT8[:, hh, 0, wsl], -1.0, tb,
                            mybir.AluOpType.mult, mybir.AluOpType.add)
                        # previous window's P3 fills PE gaps in this one
                        if prev is not None and hh < 2:
                            emit_p3(*prev, (2 * hh, 2 * hh + 1))
                    prev = (b, g)
            emit_p3(*prev, (0, 1))
            emit_p3(*prev, (2, 3))

    nc.finalize()
    return nc, nmask


_CACHE = {}


def _get_nc(window: int):
    if window not in _CACHE:
        _CACHE[window] = build_nc(window)
    return _CACHE[window]


def _split8(a):
    """e4m3 hi + unscaled lo residual planes of a [R, C] f32 array."""
    hi = a.astype(F8NP)
    lo = (a - hi.astype(np.float32)).astype(F8NP)
    return hi, lo


def _plane_pack(hi, lo, first, second):
    """Pack [R, C] planes into [128, R//128, 2, C] (p, chunk, plane, col)."""
    R, C = hi.shape
    out = np.empty((128, R // 128, 2, C), dtype=F8NP)
    sel = {"hi": hi, "lo": lo}
    out[:, :, 0, :] = sel[first].reshape(R // 128, 128, C).transpose(1, 0, 2)
    out[:, :, 1, :] = sel[second].reshape(R // 128, 128, C).transpose(1, 0, 2)
    return out


LAST_RESULTS = None


def kernel(x, w_qkv, w_o, window_size, _trace=False):
    window = int(window_size)
    nc, nmask = _get_nc(window)
    _, keys = _mask_plan(window)
    masks = _build_masks(window, keys)

    # stationary-side plane convention is (hi, lo); moving side is (lo, hi).
    xT = np.ascontiguousarray(
        x.reshape(TOK, D).T).astype(np.float32)          # [D, TOK]
    xh, xl = _split8(xT)
    x8 = _plane_pack(xh, xl, "lo", "hi")                 # moving

    inv = 1.0 / (THETA ** (np.arange(0, HD, 2, dtype=np.float64) / HD))
    freqs = np.arange(T, dtype=np.float64)[:, None] * inv[None, :]  # [T, 64]
    cosH = np.repeat(np.cos(freqs).T, 2, axis=0).astype(BF16NP)  # [128, T]
    sign = np.where(np.arange(HD) % 2 == 0, -1.0, 1.0)[:, None]
    sinH = (np.repeat(np.sin(freqs).T, 2, axis=0) * sign).astype(BF16NP)
    ident = np.eye(128).astype(BF16NP)

    in_maps = []
    for c in range(NCORES):
        wq = w_qkv[QROWS * c:QROWS * (c + 1)]
        wk = w_qkv[H * HD + HD * c: H * HD + HD * (c + 1)]
        wv = w_qkv[H * HD + G * HD + HD * c: H * HD + G * HD + HD * (c + 1)]
        W = np.ascontiguousarray(
            np.concatenate([wq, wk, wv], axis=0).T).astype(np.float32) * 64.0
        wh, wl = _split8(W)                              # [D, E]
        w8 = _plane_pack(wh, wl, "hi", "lo")             # stationary
        WO = np.ascontiguousarray(
            w_o[:, QROWS * c:QROWS * (c + 1)].T).astype(np.float32) * 64.0
        woh, wol = _split8(WO)                           # [QROWS, D]
        wo8 = _plane_pack(woh, wol, "lo", "hi")          # moving
        in_maps.append({
            "x8": x8, "w8": w8, "wo8": wo8,
            "cosH": cosH, "sinH": sinH, "masks": masks.astype(BF16NP),
            "ident": ident,
        })

    from concourse.bass_utils import run_bass_kernel_spmd
    res = run_bass_kernel_spmd(nc, in_maps, core_ids=list(range(NCORES)),
                               trace=_trace)
    global LAST_RESULTS
    LAST_RESULTS = res
    acc = res.results[0]["out"].astype(np.float32)
    for c in range(1, NCORES):
        acc = acc + res.results[c]["out"].astype(np.float32)
    # undo the 32x (oT8) * 64x (wo8) operand scaling
    return (acc / 2048.0).reshape(B, T, D)


# revision 18
# speedup vs baseline: 1.2216x; 1.0095x over previous
"""Trainium2 Bass kernel: fused QKV + RoPE + causal/windowed GQA attention + output proj.

Sharding: tensor-parallel by head across 8 cores. Core c owns Q-heads
4c..4c+3 and KV-group c (matching repeat_interleave grouping), plus the
512 w_o columns for those heads. Each core computes a full-shape partial
of the final output (contraction over its 512 attention-output dims);
the host sums the 8 partials. No device collectives.

The two big GEMMs (QKV projection P1 and output projection P3) run as
fp8e4 matmuls in DoubleRow perf mode (2 contraction k-tiles per
instruction at 0.5 cycles/row = 4x bf16 FLOP rate). Full bf16-grade
accuracy is kept with a 3-product hi/lo split per operand:
    A@B ~= Ah@Bh + Ah@Bl + Al@Bh          (Al,Bl = e4m3 residuals)
The lo planes are stored UNSCALED (e4m3 subnormals give a 2^-10 fixed
point grid there), so all 3 products share one scale and accumulate in
a single PSUM group. Operands whose scale is small (weights, sigma
1/64) are pre-scaled by 64 on the host; the 64x factors ride through
the pipeline (qkv is stored as 64x, the exp activation's scale arg
divides them back out, attention output is stored as 32x, and the host
divides the final bf16 partials by 2048).

P2 (attention) stays bf16: ST[k, q] = kT^T @ qT -> exp -> PV and
row-sum both as matmuls (softmax normalization via reciprocal +
partition_broadcast). RoPE is applied on interleaved even/odd pairs via
a DVE stream_shuffle pair swap and a sign-folded sin table.

Scheduling: P1 matmuls are emitted dc-ordered (all 6 output-row chains
advance together) so compute tracks the weight DMA stream; w8 loads
issue from the Activation HWDGE queue to run parallel with the x8
stream on SP. The batch-0 v-transposes and k-RoPE run in the shadow of
P1's second half. In P2, all 4 heads' q-RoPEs are emitted up front and
each window's output projection is emitted one window late, so the PE
always has independent work at window boundaries.
"""

import math
import sys
from contextlib import ExitStack

import numpy as np

sys.path.insert(0, "/opt/trn_rl_repo")

import ml_dtypes

BF16NP = ml_dtypes.bfloat16
F8NP = ml_dtypes.float8_e4m3

import concourse.bass as bass
import concourse.mybir as mybir
import concourse.tile as tile
from concourse import bacc

F32 = mybir.dt.float32
BF16 = mybir.dt.bfloat16
FP8 = mybir.dt.float8e4
DR = mybir.MatmulPerfMode.DoubleRow

B, T, D = 2, 2048, 4096
H, G, HD = 32, 8, 128
THETA = 10000.0
NCORES = 8
HL = H // NCORES            # 4 local q heads
TOK = B * T                 # 4096
QROWS = HL * HD             # 512 local q rows
E = QROWS + 2 * HD          # 768 local qkv rows
SCALE = 1.0 / math.sqrt(HD)
ESCALE = SCALE / 4096.0     # exp scale: q,k each carry a 64x factor

TOKG = 256                  # P1 token-group width
NTOKG = TOK // TOKG
NDC = D // 128              # 32 contraction chunks
NE = E // 128               # 6 qkv row chunks
QG = 512                    # P2 query-group width (within batch)
NQG = T // QG               # 4
NKC = T // 128              # 16 key chunks per batch
GPB = NTOKG // B            # P1 token groups per batch


def _mask_plan(window: int):
    """Per (qgroup, kchunk): 'skip', 'full', or a mask-key (delta-based)."""
    plan = {}
    keys = {}
    for g in range(NQG):
        for kc in range(NKC):
            i_min, i_max = QG * g, QG * g + QG - 1
            j_min, j_max = 128 * kc, 128 * kc + 127
            if j_min > i_max or (i_min - j_max) >= window:
                plan[(g, kc)] = ("skip", None)
            elif j_max <= i_min and (i_max - j_min) < window:
                plan[(g, kc)] = ("full", None)
            else:
                key = QG * g - 128 * kc
                if key not in keys:
                    keys[key] = len(keys)
                plan[(g, kc)] = ("mask", keys[key])
    return plan, keys


def _build_masks(window: int, keys: dict) -> np.ndarray:
    n = max(1, len(keys))
    m = np.zeros((n, 128, QG), dtype=np.float32)  # cast to bf16 in kernel()
    for key, idx in keys.items():
        # i = key + 128*kc ... i - j = key + qq - kk
        qq = np.arange(QG)[None, :]
        kk = np.arange(128)[:, None]
        diff = key + qq - kk          # i - j
        vis = (diff >= 0) & (diff < window)
        m[idx] = np.where(vis, 1.0, 0.0)
    return m


PAIRSWAP = [i ^ 1 for i in range(32)]


def _rope_ops(nc, pool, dst, src, cos_ap, sin_ap):
    """Interleaved-pair RoPE: dst = src*cos + pairswap(src)*signed_sin.

    cos_ap rows (2i, 2i+1) hold cos_i; sin_ap rows hold (-sin_i, +sin_i).
    src may alias dst (in-place).
    """
    W = dst.shape[-1]
    sw = pool.tile([128, W], BF16, tag="rope_sw")
    tmp = pool.tile([128, W], BF16, tag="rope_tmp")
    qc = pool.tile([128, W], BF16, tag="rope_qc")
    mult = mybir.AluOpType.mult
    nc.vector.stream_shuffle(sw, src, PAIRSWAP)
    nc.vector.tensor_tensor(tmp, sw, sin_ap, mult)
    nc.vector.tensor_tensor(qc, src, cos_ap, mult)
    nc.vector.tensor_tensor(dst, qc, tmp, mybir.AluOpType.add)


def build_nc(window: int):
    plan, keys = _mask_plan(window)
    nmask = max(1, len(keys))

    nc = bacc.Bacc()
    x8_d = nc.dram_tensor("x8", [128, NDC, 2, TOK], FP8, kind="ExternalInput")
    w8_d = nc.dram_tensor("w8", [128, NDC, 2, E], FP8, kind="ExternalInput")
    wo8_d = nc.dram_tensor("wo8", [128, HL, 2, D], FP8, kind="ExternalInput")
    cos_d = nc.dram_tensor("cosH", [128, T], BF16, kind="ExternalInput")
    sin_d = nc.dram_tensor("sinH", [128, T], BF16, kind="ExternalInput")
    masks_d = nc.dram_tensor("masks", [nmask, 128, QG], BF16, kind="ExternalInput")
    ident_d = nc.dram_tensor("ident", [128, 128], BF16, kind="ExternalInput")
    out_d = nc.dram_tensor("out", [TOK, D], BF16, kind="ExternalOutput")

    with ExitStack() as octx:
        tc = octx.enter_context(tile.TileContext(nc))
        qkvp = octx.enter_context(tc.tile_pool(name="qkvT", bufs=1))
        qkvT_sb = [qkvp.tile([128, TOK], BF16, tag=f"qkv{e}", name=f"qkv{e}")
                   for e in range(NE)]
        opool = octx.enter_context(tc.tile_pool(name="outT", bufs=1))
        oT8 = opool.tile([128, HL, 2, TOK], FP8, name="oT8")
        kvp = octx.enter_context(tc.tile_pool(name="kv", bufs=1))
        vsb = kvp.tile([128, TOK // 128, 128], BF16, tag="v")
        cos_sb = kvp.tile([128, T], BF16, tag="cos")
        sin_sb = kvp.tile([128, T], BF16, tag="sin")
        ones_sb = kvp.tile([128, 1], BF16, tag="ones")
        mask_sb = kvp.tile([128, nmask, QG], BF16, tag="masks")
        ident = kvp.tile([128, 128], BF16, tag="ident")

        nc.sync.dma_start(out=ident, in_=ident_d[:])
        nc.sync.dma_start(out=cos_sb, in_=cos_d[:])
        nc.sync.dma_start(out=sin_sb, in_=sin_d[:])
        nc.sync.dma_start(
            out=mask_sb, in_=masks_d[:].rearrange("n p q -> p n q"))
        # rowsum weights of 2.0 fold the 1/2 of the 32x output scale into
        # the reciprocal: rrec = 1/(2Z)
        nc.vector.memset(ones_sb, 2.0)

        ksb = qkvT_sb[HL]
        vT = qkvT_sb[HL + 1]
        qpool = octx.enter_context(tc.tile_pool(name="q2", bufs=5))
        ropep = octx.enter_context(tc.tile_pool(name="ropes", bufs=2))
        rope0 = []

        # ---------------- P1: qkvT(64x) = (64 w)^T @ x, fp8 DoubleRow ---------
        with ExitStack() as ctx:
            wpool = ctx.enter_context(tc.tile_pool(name="w1", bufs=1))
            xpool = ctx.enter_context(tc.tile_pool(name="x1", bufs=2))
            ppool = ctx.enter_context(tc.tile_pool(name="ps1", bufs=1, space="PSUM"))
            pt0 = ctx.enter_context(tc.tile_pool(name="p2aps", bufs=2, space="PSUM"))

            # weights stream on the Activation HWDGE queue, x on SP: the two
            # queues run in parallel and P1's dc-ordered chains track them.
            wsb = wpool.tile([128, NDC, 2, E], FP8)
            for dc4 in range(NDC // 4):
                nc.scalar.dma_start(out=wsb[:, 4 * dc4:4 * (dc4 + 1)],
                                    in_=w8_d[:, 4 * dc4:4 * (dc4 + 1)])
            for g in range(NTOKG):
                xsb = xpool.tile([128, NDC, 2, TOKG], FP8, tag="xslab")
                for dq in range(4):
                    nc.sync.dma_start(
                        out=xsb[:, dq * 8:(dq + 1) * 8],
                        in_=x8_d[:, dq * 8:(dq + 1) * 8, :,
                                 g * TOKG:(g + 1) * TOKG])
                pss = [ppool.tile([128, TOKG], F32, tag=f"p1_{e}",
                                  name=f"p1_{e}") for e in range(NE)]
                for p in range(NDC // 2):
                    for e in range(NE):
                        es = slice(e * 128, (e + 1) * 128)
                        # main: (w_hi, x_hi) over the chunk pair
                        nc.tensor.matmul(
                            pss[e],
                            lhsT=wsb[:, 2 * p:2 * p + 2, 0, es],
                            rhs=xsb[:, 2 * p:2 * p + 2, 1, :],
                            start=(p == 0), stop=False, perf_mode=DR)
                        # corr: (w_hi x_lo) + (w_lo x_hi) per chunk
                        for dc in (2 * p, 2 * p + 1):
                            nc.tensor.matmul(
                                pss[e],
                                lhsT=wsb[:, dc, 0:2, es],
                                rhs=xsb[:, dc, 0:2, :],
                                start=False,
                                stop=(dc == NDC - 1), perf_mode=DR)
                for e in range(NE):
                    nc.scalar.copy(
                        qkvT_sb[e][:, g * TOKG:(g + 1) * TOKG], pss[e])

                # batch b's K/V post-processing in the shadow of P1's tail
                if (g + 1) % GPB == 0:
                    b = (g + 1) // GPB - 1
                    for tc32 in range(b * NKC, (b + 1) * NKC):
                        pst = pt0.tile([128, 128], BF16, tag="tr")
                        nc.tensor.transpose(
                            pst, vT[:, tc32 * 128:(tc32 + 1) * 128], ident)
                        nc.scalar.copy(vsb[:, tc32, :], pst)
                    for sl in range(T // QG):
                        ks = ksb[:, b * T + sl * QG: b * T + (sl + 1) * QG]
                        _rope_ops(nc, ropep, ks, ks,
                                  cos_sb[:, sl * QG:(sl + 1) * QG],
                                  sin_sb[:, sl * QG:(sl + 1) * QG])
                    if b == 0:
                        # first attention window's q-RoPEs: DVE is idle here
                        for hh in range(HL):
                            qsb = qpool.tile([128, QG], BF16, tag="q",
                                             name="qsb")
                            _rope_ops(nc, ropep, qsb, qkvT_sb[hh][:, 0:QG],
                                      cos_sb[:, 0:QG], sin_sb[:, 0:QG])
                            rope0.append(qsb)

        # ---------------- P2 attention + interleaved P3 ----------------
        wopool = octx.enter_context(tc.tile_pool(name="wo", bufs=1))
        wo8 = wopool.tile([128, HL, 2, D], FP8, name="wo8")
        for dc in range(HL):
            # ACT HWDGE queue is idle right after P1; arrives before first P3
            nc.scalar.dma_start(out=wo8[:, dc], in_=wo8_d[:, dc])
        with ExitStack() as ctx:
            spool = ctx.enter_context(tc.tile_pool(name="sc2", bufs=4))
            estp = ctx.enter_context(tc.tile_pool(name="est", bufs=6))
            panp = ctx.enter_context(tc.tile_pool(name="pan", bufs=2))
            stps = ctx.enter_context(tc.tile_pool(name="stps", bufs=3, space="PSUM"))
            rps = ctx.enter_context(tc.tile_pool(name="rps", bufs=1, space="PSUM"))
            ops = ctx.enter_context(tc.tile_pool(name="ops", bufs=2, space="PSUM"))
            pps = ctx.enter_context(tc.tile_pool(name="ps3", bufs=2, space="PSUM"))

            def emit_p3(bw, gw, tlocs):
                # fp8 DoubleRow output projection for finished window (bw,gw)
                for tloc in tlocs:
                    tch = (bw * T + gw * QG) // 128 + tloc
                    tsl = slice(tch * 128, (tch + 1) * 128)
                    panel = panp.tile([128, D], BF16, tag="panel")
                    for et in range(D // 512):
                        ps = pps.tile([128, 512], F32, tag="p3")
                        esl = slice(et * 512, (et + 1) * 512)
                        for p in range(HL // 2):
                            nc.tensor.matmul(
                                ps,
                                lhsT=oT8[:, 2 * p:2 * p + 2, 0, tsl],
                                rhs=wo8[:, 2 * p:2 * p + 2, 1, esl],
                                start=(p == 0), stop=False, perf_mode=DR)
                        for dc in range(HL):
                            nc.tensor.matmul(
                                ps,
                                lhsT=oT8[:, dc, 0:2, tsl],
                                rhs=wo8[:, dc, 0:2, esl],
                                start=False, stop=(dc == HL - 1),
                                perf_mode=DR)
                        if et % 2 == 0:
                            nc.scalar.copy(panel[:, esl], ps)
                        else:
                            nc.vector.tensor_scalar_mul(panel[:, esl], ps, 1.0)
                        if et == D // 512 // 2 - 1:
                            nc.sync.dma_start(out=out_d[tsl, :D // 2],
                                              in_=panel[:, :D // 2])
                    nc.sync.dma_start(out=out_d[tsl, D // 2:],
                                      in_=panel[:, D // 2:])

            prev = None
            for b in range(B):
                for g in range(NQG):
                    # q-RoPE for all 4 heads up front: DVE leads the PE
                    if b == 0 and g == 0:
                        qsbs = rope0
                    else:
                        qsbs = []
                        for hh in range(HL):
                            qsb = qpool.tile([128, QG], BF16, tag="q",
                                             name="qsb")
                            _rope_ops(nc, ropep, qsb,
                                      qkvT_sb[hh][:, b * T + g * QG:
                                                  b * T + (g + 1) * QG],
                                      cos_sb[:, g * QG:(g + 1) * QG],
                                      sin_sb[:, g * QG:(g + 1) * QG])
                            qsbs.append(qsb)
                    vis = [(kc, plan[(g, kc)]) for kc in range(NKC)
                           if plan[(g, kc)][0] != "skip"]
                    for hh in range(HL):
                        qsb = qsbs[hh]
                        r_ps = rps.tile([1, QG], F32, tag="r")
                        o_ps = ops.tile([128, QG], F32, tag="o")
                        for idx, (kc, (kind, mid)) in enumerate(vis):
                            # visible query subrange of this key chunk:
                            # qq >= -aoff (causal), qq < w - aoff + 127
                            aoff = QG * g - 128 * kc
                            qlo = max(0, -aoff)
                            qhi = min(QG, window - aoff + 127)
                            qsl = slice(qlo, qhi)
                            st = stps.tile([128, QG], F32, tag="st")
                            nc.tensor.matmul(
                                st[:, qsl],
                                lhsT=ksb[:, b * T + kc * 128:
                                         b * T + (kc + 1) * 128],
                                rhs=qsb[:, qsl],
                                start=True, stop=True)
                            est = estp.tile([128, QG], BF16, tag="est")
                            nc.scalar.activation(
                                est[:, qsl], st[:, qsl],
                                mybir.ActivationFunctionType.Exp,
                                scale=ESCALE)
                            if kind == "mask":
                                nc.vector.tensor_tensor(
                                    est[:, qsl], est[:, qsl],
                                    mask_sb[:, mid, qsl],
                                    mybir.AluOpType.mult)
                            last = idx == len(vis) - 1
                            nc.tensor.matmul(
                                r_ps[:, qsl], lhsT=ones_sb,
                                rhs=est[:, qsl],
                                start=(idx == 0), stop=last)
                            nc.tensor.matmul(
                                o_ps[:, qsl],
                                lhsT=vsb[:, b * NKC + kc, :],
                                rhs=est[:, qsl],
                                start=(idx == 0), stop=last)
                        rrec = spool.tile([1, QG], F32, tag="rrec")
                        nc.vector.reciprocal(rrec, r_ps)
                        rb = spool.tile([128, QG], F32, tag="rb")
                        nc.gpsimd.partition_broadcast(rb, rrec)
                        # tb = o_ps/(2Z) = 32*(o/Z); split into fp8 hi+lo
                        wsl = slice(b * T + g * QG, b * T + (g + 1) * QG)
                        tb = spool.tile([128, QG], BF16, tag="tb")
                        nc.vector.tensor_tensor(
                            tb, o_ps, rb, mybir.AluOpType.mult)
                        nc.scalar.copy(oT8[:, hh, 0, wsl], tb)
                        nc.vector.scalar_tensor_tensor(
                            oT8[:, hh, 1, wsl], oT8[:, hh, 0, wsl], -1.0, tb,
                            mybir.AluOpType.mult, mybir.AluOpType.add)
                        # previous window's P3 fills PE gaps in this one
                        if prev is not None and hh < 2:
                            emit_p3(*prev, (2 * hh, 2 * hh + 1))
                    prev = (b, g)
            emit_p3(*prev, (0, 1))
            emit_p3(*prev, (2, 3))

    nc.finalize()
    return nc, nmask


_CACHE = {}


def _get_nc(window: int):
    if window not in _CACHE:
        _CACHE[window] = build_nc(window)
    return _CACHE[window]


def _split8(a):
    """e4m3 hi + unscaled lo residual planes of a [R, C] f32 array."""
    hi = a.astype(F8NP)
    lo = (a - hi.astype(np.float32)).astype(F8NP)
    return hi, lo


def _plane_pack(hi, lo, first, second):
    """Pack [R, C] planes into [128, R//128, 2, C] (p, chunk, plane, col)."""
    R, C = hi.shape
    out = np.empty((128, R // 128, 2, C), dtype=F8NP)
    sel = {"hi": hi, "lo": lo}
    out[:, :, 0, :] = sel[first].reshape(R // 128, 128, C).transpose(1, 0, 2)
    out[:, :, 1, :] = sel[second].reshape(R // 128, 128, C).transpose(1, 0, 2)
    return out


LAST_RESULTS = None


def kernel(x, w_qkv, w_o, window_size, _trace=False):
    window = int(window_size)
    nc, nmask = _get_nc(window)
    _, keys = _mask_plan(window)
    masks = _build_masks(window, keys)

    # stationary-side plane convention is (hi, lo); moving side is (lo, hi).
    xT = np.ascontiguousarray(
        x.reshape(TOK, D).T).astype(np.float32)          # [D, TOK]
    xh, xl = _split8(xT)
    x8 = _plane_pack(xh, xl, "lo", "hi")                 # moving

    inv = 1.0 / (THETA ** (np.arange(0, HD, 2, dtype=np.float64) / HD))
    freqs = np.arange(T, dtype=np.float64)[:, None] * inv[None, :]  # [T, 64]
    cosH = np.repeat(np.cos(freqs).T, 2, axis=0).astype(BF16NP)  # [128, T]
    sign = np.where(np.arange(HD) % 2 == 0, -1.0, 1.0)[:, None]
    sinH = (np.repeat(np.sin(freqs).T, 2, axis=0) * sign).astype(BF16NP)
    ident = np.eye(128).astype(BF16NP)

    in_maps = []
    for c in range(NCORES):
        wq = w_qkv[QROWS * c:QROWS * (c + 1)]
        wk = w_qkv[H * HD + HD * c: H * HD + HD * (c + 1)]
        wv = w_qkv[H * HD + G * HD + HD * c: H * HD + G * HD + HD * (c + 1)]
        W = np.ascontiguousarray(
            np.concatenate([wq, wk, wv], axis=0).T).astype(np.float32) * 64.0
        wh, wl = _split8(W)                              # [D, E]
        w8 = _plane_pack(wh, wl, "hi", "lo")             # stationary
        WO = np.ascontiguousarray(
            w_o[:, QROWS * c:QROWS * (c + 1)].T).astype(np.float32) * 64.0
        woh, wol = _split8(WO)                           # [QROWS, D]
        wo8 = _plane_pack(woh, wol, "lo", "hi")          # moving
        in_maps.append({
            "x8": x8, "w8": w8, "wo8": wo8,
            "cosH": cosH, "sinH": sinH, "masks": masks.astype(BF16NP),
            "ident": ident,
        })

    from concourse.bass_utils import run_bass_kernel_spmd
    res = run_bass_kernel_spmd(nc, in_maps, core_ids=list(range(NCORES)),
                               trace=_trace)
    global LAST_RESULTS
    LAST_RESULTS = res
    acc = res.results[0]["out"].astype(np.float32)
    for c in range(1, NCORES):
        acc = acc + res.results[c]["out"].astype(np.float32)
    # undo the 32x (oT8) * 64x (wo8) operand scaling
    return (acc / 2048.0).reshape(B, T, D)


# revision 19
# speedup vs baseline: 1.2322x; 1.0087x over previous
"""Trainium2 Bass kernel: fused QKV + RoPE + causal/windowed GQA attention + output proj.

Sharding: tensor-parallel by head across 8 cores. Core c owns Q-heads
4c..4c+3 and KV-group c (matching repeat_interleave grouping), plus the
512 w_o columns for those heads. Each core computes a full-shape partial
of the final output (contraction over its 512 attention-output dims);
the host sums the 8 partials. No device collectives.

The two big GEMMs (QKV projection P1 and output projection P3) run as
fp8e4 matmuls in DoubleRow perf mode (2 contraction k-tiles per
instruction at 0.5 cycles/row = 4x bf16 FLOP rate). Full bf16-grade
accuracy is kept with a 3-product hi/lo split per operand:
    A@B ~= Ah@Bh + Ah@Bl + Al@Bh          (Al,Bl = e4m3 residuals)
The lo planes are stored UNSCALED (e4m3 subnormals give a 2^-10 fixed
point grid there), so all 3 products share one scale and accumulate in
a single PSUM group. Operands whose scale is small (weights, sigma
1/64) are pre-scaled by 64 on the host; the 64x factors ride through
the pipeline (qkv is stored as 64x, the exp activation's scale arg
divides them back out, attention output is stored as 32x, and the host
divides the final bf16 partials by 2048).

P2 (attention) stays bf16: ST[k, q] = kT^T @ qT -> exp -> PV and
row-sum both as matmuls (softmax normalization via reciprocal +
partition_broadcast). RoPE is applied on interleaved even/odd pairs via
a DVE stream_shuffle pair swap and a sign-folded sin table.

Scheduling notes. The Tile dependency tracker is tile-granular (a read
waits on ALL earlier writers of the tile), so tiles are sized to the
wavefront: weights arrive as eight 4-chunk tiles on the Activation
HWDGE queue (parallel to the x stream on SP), x token-slabs arrive as
four quarter tiles, qkv/v live in per-batch tiles so batch-0 post
processing (v transpose, k/q RoPE) runs in the shadow of P1's second
half, and attention outputs live in per-window tiles so next window's
writes never false-depend on this window's P3 reads. P1 matmuls are
emitted dc-ordered (all 6 output-row chains advance together) to track
the weight stream; each window's output projection is emitted one
window late, one token-chunk before each head, so the PE has
independent work while a head's exp chain spins up.
"""

import math
import sys
from contextlib import ExitStack

import numpy as np

sys.path.insert(0, "/opt/trn_rl_repo")

import ml_dtypes

BF16NP = ml_dtypes.bfloat16
F8NP = ml_dtypes.float8_e4m3

import concourse.bass as bass
import concourse.mybir as mybir
import concourse.tile as tile
from concourse import bacc

F32 = mybir.dt.float32
BF16 = mybir.dt.bfloat16
FP8 = mybir.dt.float8e4
DR = mybir.MatmulPerfMode.DoubleRow

B, T, D = 2, 2048, 4096
H, G, HD = 32, 8, 128
THETA = 10000.0
NCORES = 8
HL = H // NCORES            # 4 local q heads
TOK = B * T                 # 4096
QROWS = HL * HD             # 512 local q rows
E = QROWS + 2 * HD          # 768 local qkv rows
SCALE = 1.0 / math.sqrt(HD)
ESCALE = SCALE / 4096.0     # exp scale: q,k each carry a 64x factor

TOKG = 256                  # P1 token-group width
NTOKG = TOK // TOKG
NDC = D // 128              # 32 contraction chunks
NWT = 8                     # weight tiles (4 chunks each)
DCW = NDC // NWT
NE = E // 128               # 6 qkv row chunks
QG = 512                    # P2 query-group width (within batch)
NQG = T // QG               # 4
NKC = T // 128              # 16 key chunks per batch
GPB = NTOKG // B            # P1 token groups per batch


def _mask_plan(window: int):
    """Per (qgroup, kchunk): 'skip', 'full', or a mask-key (delta-based)."""
    plan = {}
    keys = {}
    for g in range(NQG):
        for kc in range(NKC):
            i_min, i_max = QG * g, QG * g + QG - 1
            j_min, j_max = 128 * kc, 128 * kc + 127
            if j_min > i_max or (i_min - j_max) >= window:
                plan[(g, kc)] = ("skip", None)
            elif j_max <= i_min and (i_max - j_min) < window:
                plan[(g, kc)] = ("full", None)
            else:
                key = QG * g - 128 * kc
                if key not in keys:
                    keys[key] = len(keys)
                plan[(g, kc)] = ("mask", keys[key])
    return plan, keys


def _build_masks(window: int, keys: dict) -> np.ndarray:
    n = max(1, len(keys))
    m = np.zeros((n, 128, QG), dtype=np.float32)  # cast to bf16 in kernel()
    for key, idx in keys.items():
        # i = key + 128*kc ... i - j = key + qq - kk
        qq = np.arange(QG)[None, :]
        kk = np.arange(128)[:, None]
        diff = key + qq - kk          # i - j
        vis = (diff >= 0) & (diff < window)
        m[idx] = np.where(vis, 1.0, 0.0)
    return m


PAIRSWAP = [i ^ 1 for i in range(32)]


def _rope_ops(nc, pool, dst, src, cos_ap, sin_ap):
    """Interleaved-pair RoPE: dst = src*cos + pairswap(src)*signed_sin.

    cos_ap rows (2i, 2i+1) hold cos_i; sin_ap rows hold (-sin_i, +sin_i).
    src may alias dst (in-place).
    """
    W = dst.shape[-1]
    sw = pool.tile([128, W], BF16, tag="rope_sw")
    tmp = pool.tile([128, W], BF16, tag="rope_tmp")
    qc = pool.tile([128, W], BF16, tag="rope_qc")
    mult = mybir.AluOpType.mult
    nc.vector.stream_shuffle(sw, src, PAIRSWAP)
    nc.vector.tensor_tensor(tmp, sw, sin_ap, mult)
    nc.vector.tensor_tensor(qc, src, cos_ap, mult)
    nc.vector.tensor_tensor(dst, qc, tmp, mybir.AluOpType.add)


def build_nc(window: int):
    plan, keys = _mask_plan(window)
    nmask = max(1, len(keys))

    nc = bacc.Bacc()
    x8_d = nc.dram_tensor("x8", [128, NDC, 2, TOK], FP8, kind="ExternalInput")
    w8_d = nc.dram_tensor("w8", [128, NDC, 2, E], FP8, kind="ExternalInput")
    wo8_d = nc.dram_tensor("wo8", [128, HL, 2, D], FP8, kind="ExternalInput")
    cos_d = nc.dram_tensor("cosH", [128, T], BF16, kind="ExternalInput")
    sin_d = nc.dram_tensor("sinH", [128, T], BF16, kind="ExternalInput")
    masks_d = nc.dram_tensor("masks", [nmask, 128, QG], BF16, kind="ExternalInput")
    ident_d = nc.dram_tensor("ident", [128, 128], BF16, kind="ExternalInput")
    out_d = nc.dram_tensor("out", [TOK, D], BF16, kind="ExternalOutput")

    with ExitStack() as octx:
        tc = octx.enter_context(tile.TileContext(nc))
        qkvp = octx.enter_context(tc.tile_pool(name="qkvT", bufs=1))
        # per-batch qkv tiles: batch-b consumers only wait on batch-b writes
        qkvT_sb = [[qkvp.tile([128, T], BF16, tag=f"qkv{b}_{e}",
                              name=f"qkv{b}_{e}") for e in range(NE)]
                   for b in range(B)]
        opool = octx.enter_context(tc.tile_pool(name="outT", bufs=3))
        kvp = octx.enter_context(tc.tile_pool(name="kv", bufs=1))
        vsb = [kvp.tile([128, NKC, 128], BF16, tag=f"v{b}", name=f"v{b}")
               for b in range(B)]
        cos_sb = kvp.tile([128, T], BF16, tag="cos")
        sin_sb = kvp.tile([128, T], BF16, tag="sin")
        ones_sb = kvp.tile([128, 1], BF16, tag="ones")
        mask_sb = kvp.tile([128, nmask, QG], BF16, tag="masks")
        ident = kvp.tile([128, 128], BF16, tag="ident")

        nc.sync.dma_start(out=ident, in_=ident_d[:])
        nc.sync.dma_start(out=cos_sb, in_=cos_d[:])
        nc.sync.dma_start(out=sin_sb, in_=sin_d[:])
        nc.sync.dma_start(
            out=mask_sb, in_=masks_d[:].rearrange("n p q -> p n q"))
        # rowsum weights of 2.0 fold the 1/2 of the 32x output scale into
        # the reciprocal: rrec = 1/(2Z)
        nc.vector.memset(ones_sb, 2.0)

        qpool = octx.enter_context(tc.tile_pool(name="q2", bufs=5))
        ropep = octx.enter_context(tc.tile_pool(name="ropes", bufs=2))
        rope0 = []

        # ---------------- P1: qkvT(64x) = (64 w)^T @ x, fp8 DoubleRow ---------
        with ExitStack() as ctx:
            wpool = ctx.enter_context(tc.tile_pool(name="w1", bufs=1))
            xpool = ctx.enter_context(tc.tile_pool(name="x1", bufs=2))
            ppool = ctx.enter_context(tc.tile_pool(name="ps1", bufs=1, space="PSUM"))
            pt0 = ctx.enter_context(tc.tile_pool(name="p2aps", bufs=2, space="PSUM"))

            # weight stream on the Activation HWDGE queue, x stream on SP:
            # the queues run in parallel and the dc-ordered chains below
            # consume tiles as they land.
            wsb = []
            for wt in range(NWT):
                w = wpool.tile([128, DCW, 2, E], FP8, tag=f"w{wt}",
                               name=f"w{wt}")
                nc.scalar.dma_start(out=w, in_=w8_d[:, wt * DCW:(wt + 1) * DCW])
                wsb.append(w)
            for g in range(NTOKG):
                b, col = g // GPB, (g % GPB) * TOKG
                xq = []
                for dq in range(4):
                    xt = xpool.tile([128, NDC // 4, 2, TOKG], FP8,
                                    tag=f"xq{dq}", name=f"xq{dq}")
                    nc.sync.dma_start(
                        out=xt, in_=x8_d[:, dq * 8:(dq + 1) * 8, :,
                                         g * TOKG:(g + 1) * TOKG])
                    xq.append(xt)
                pss = [ppool.tile([128, TOKG], F32, tag=f"p1_{e}",
                                  name=f"p1_{e}") for e in range(NE)]
                for p in range(NDC // 2):
                    wt, wo_ = wsb[p // 2], (p % 2) * 2      # tile, dc offset
                    xt, xo = xq[p // 4], (p % 4) * 2
                    for e in range(NE):
                        es = slice(e * 128, (e + 1) * 128)
                        # main: (w_hi, x_hi) over the chunk pair
                        nc.tensor.matmul(
                            pss[e],
                            lhsT=wt[:, wo_:wo_ + 2, 0, es],
                            rhs=xt[:, xo:xo + 2, 1, :],
                            start=(p == 0), stop=False, perf_mode=DR)
                        # corr: (w_hi x_lo) + (w_lo x_hi) per chunk
                        for i in range(2):
                            nc.tensor.matmul(
                                pss[e],
                                lhsT=wt[:, wo_ + i, 0:2, es],
                                rhs=xt[:, xo + i, 0:2, :],
                                start=False,
                                stop=(p == NDC // 2 - 1 and i == 1),
                                perf_mode=DR)
                for e in range(NE):
                    nc.scalar.copy(qkvT_sb[b][e][:, col:col + TOKG], pss[e])

                # batch b's K/V post-processing in the shadow of P1's tail
                if (g + 1) % GPB == 0:
                    for tc32 in range(NKC):
                        pst = pt0.tile([128, 128], BF16, tag="tr")
                        nc.tensor.transpose(
                            pst, qkvT_sb[b][HL + 1][:, tc32 * 128:
                                                    (tc32 + 1) * 128], ident)
                        nc.scalar.copy(vsb[b][:, tc32, :], pst)
                    for sl in range(T // QG):
                        ks = qkvT_sb[b][HL][:, sl * QG:(sl + 1) * QG]
                        _rope_ops(nc, ropep, ks, ks,
                                  cos_sb[:, sl * QG:(sl + 1) * QG],
                                  sin_sb[:, sl * QG:(sl + 1) * QG])
                    if b == 0:
                        # first attention window's q-RoPEs: DVE is idle here
                        for hh in range(HL):
                            qsb = qpool.tile([128, QG], BF16, tag="q",
                                             name="qsb")
                            _rope_ops(nc, ropep, qsb,
                                      qkvT_sb[0][hh][:, 0:QG],
                                      cos_sb[:, 0:QG], sin_sb[:, 0:QG])
                            rope0.append(qsb)

        # ---------------- P2 attention + interleaved P3 ----------------
        wopool = octx.enter_context(tc.tile_pool(name="wo", bufs=1))
        wo8 = wopool.tile([128, HL, 2, D], FP8, name="wo8")
        for dc in range(HL):
            # ACT HWDGE queue is idle right after P1; arrives before first P3
            nc.scalar.dma_start(out=wo8[:, dc], in_=wo8_d[:, dc])
        with ExitStack() as ctx:
            spool = ctx.enter_context(tc.tile_pool(name="sc2", bufs=4))
            estp = ctx.enter_context(tc.tile_pool(name="est", bufs=6))
            panp = ctx.enter_context(tc.tile_pool(name="pan", bufs=2))
            stps = ctx.enter_context(tc.tile_pool(name="stps", bufs=3, space="PSUM"))
            rps = ctx.enter_context(tc.tile_pool(name="rps", bufs=1, space="PSUM"))
            ops = ctx.enter_context(tc.tile_pool(name="ops", bufs=2, space="PSUM"))
            pps = ctx.enter_context(tc.tile_pool(name="ps3", bufs=2, space="PSUM"))

            def emit_p3(ot, bw, gw, tloc):
                # fp8 DoubleRow output projection, one 128-token chunk of the
                # finished window (bw,gw)
                tch = (bw * T + gw * QG) // 128 + tloc
                tsl = slice(tch * 128, (tch + 1) * 128)
                osl = slice(tloc * 128, (tloc + 1) * 128)
                panel = panp.tile([128, D], BF16, tag="panel")
                for et in range(D // 512):
                    ps = pps.tile([128, 512], F32, tag="p3")
                    esl = slice(et * 512, (et + 1) * 512)
                    for p in range(HL // 2):
                        nc.tensor.matmul(
                            ps,
                            lhsT=ot[:, 2 * p:2 * p + 2, 0, osl],
                            rhs=wo8[:, 2 * p:2 * p + 2, 1, esl],
                            start=(p == 0), stop=False, perf_mode=DR)
                    for dc in range(HL):
                        nc.tensor.matmul(
                            ps,
                            lhsT=ot[:, dc, 0:2, osl],
                            rhs=wo8[:, dc, 0:2, esl],
                            start=False, stop=(dc == HL - 1),
                            perf_mode=DR)
                    if et % 2 == 0:
                        nc.scalar.copy(panel[:, esl], ps)
                    else:
                        nc.vector.tensor_scalar_mul(panel[:, esl], ps, 1.0)
                    if et == D // 512 // 2 - 1:
                        nc.sync.dma_start(out=out_d[tsl, :D // 2],
                                          in_=panel[:, :D // 2])
                nc.sync.dma_start(out=out_d[tsl, D // 2:],
                                  in_=panel[:, D // 2:])

            prev = None
            for b in range(B):
                for g in range(NQG):
                    # q-RoPE for all 4 heads up front: DVE leads the PE
                    if b == 0 and g == 0:
                        qsbs = rope0
                    else:
                        qsbs = []
                        for hh in range(HL):
                            qsb = qpool.tile([128, QG], BF16, tag="q",
                                             name="qsb")
                            _rope_ops(nc, ropep, qsb,
                                      qkvT_sb[b][hh][:, g * QG:(g + 1) * QG],
                                      cos_sb[:, g * QG:(g + 1) * QG],
                                      sin_sb[:, g * QG:(g + 1) * QG])
                            qsbs.append(qsb)
                    ot8 = opool.tile([128, HL, 2, QG], FP8, tag="ot8",
                                     name="ot8")
                    vis = [(kc, plan[(g, kc)]) for kc in range(NKC)
                           if plan[(g, kc)][0] != "skip"]
                    for hh in range(HL):
                        # previous window's P3 gives the PE independent work
                        # while this head's exp chain spins up
                        if prev is not None:
                            emit_p3(*prev, hh)
                        qsb = qsbs[hh]
                        r_ps = rps.tile([1, QG], F32, tag="r")
                        o_ps = ops.tile([128, QG], F32, tag="o")
                        for idx, (kc, (kind, mid)) in enumerate(vis):
                            # visible query subrange of this key chunk:
                            # qq >= -aoff (causal), qq < w - aoff + 127
                            aoff = QG * g - 128 * kc
                            qlo = max(0, -aoff)
                            qhi = min(QG, window - aoff + 127)
                            qsl = slice(qlo, qhi)
                            st = stps.tile([128, QG], F32, tag="st")
                            nc.tensor.matmul(
                                st[:, qsl],
                                lhsT=qkvT_sb[b][HL][:, kc * 128:
                                                    (kc + 1) * 128],
                                rhs=qsb[:, qsl],
                                start=True, stop=True)
                            est = estp.tile([128, QG], BF16, tag="est")
                            nc.scalar.activation(
                                est[:, qsl], st[:, qsl],
                                mybir.ActivationFunctionType.Exp,
                                scale=ESCALE)
                            if kind == "mask":
                                nc.vector.tensor_tensor(
                                    est[:, qsl], est[:, qsl],
                                    mask_sb[:, mid, qsl],
                                    mybir.AluOpType.mult)
                            last = idx == len(vis) - 1
                            nc.tensor.matmul(
                                r_ps[:, qsl], lhsT=ones_sb,
                                rhs=est[:, qsl],
                                start=(idx == 0), stop=last)
                            nc.tensor.matmul(
                                o_ps[:, qsl],
                                lhsT=vsb[b][:, kc, :],
                                rhs=est[:, qsl],
                                start=(idx == 0), stop=last)
                        rrec = spool.tile([1, QG], F32, tag="rrec")
                        nc.vector.reciprocal(rrec, r_ps)
                        rb = spool.tile([128, QG], F32, tag="rb")
                        nc.gpsimd.partition_broadcast(rb, rrec)
                        # tb = o_ps/(2Z) = 32*(o/Z); split into fp8 hi+lo
                        tb = spool.tile([128, QG], BF16, tag="tb")
                        nc.vector.tensor_tensor(
                            tb, o_ps, rb, mybir.AluOpType.mult)
                        nc.scalar.copy(ot8[:, hh, 0, :], tb)
                        nc.vector.scalar_tensor_tensor(
                            ot8[:, hh, 1, :], ot8[:, hh, 0, :], -1.0, tb,
                            mybir.AluOpType.mult, mybir.AluOpType.add)
                    prev = (ot8, b, g)
            for tloc in range(QG // 128):
                emit_p3(*prev, tloc)

    nc.finalize()
    return nc, nmask


_CACHE = {}


def _get_nc(window: int):
    if window not in _CACHE:
        _CACHE[window] = build_nc(window)
    return _CACHE[window]


def _split8(a):
    """e4m3 hi + unscaled lo residual planes of a [R, C] f32 array."""
    hi = a.astype(F8NP)
    lo = (a - hi.astype(np.float32)).astype(F8NP)
    return hi, lo


def _plane_pack(hi, lo, first, second):
    """Pack [R, C] planes into [128, R//128, 2, C] (p, chunk, plane, col)."""
    R, C = hi.shape
    out = np.empty((128, R // 128, 2, C), dtype=F8NP)
    sel = {"hi": hi, "lo": lo}
    out[:, :, 0, :] = sel[first].reshape(R // 128, 128, C).transpose(1, 0, 2)
    out[:, :, 1, :] = sel[second].reshape(R // 128, 128, C).transpose(1, 0, 2)
    return out


LAST_RESULTS = None


def kernel(x, w_qkv, w_o, window_size, _trace=False):
    window = int(window_size)
    nc, nmask = _get_nc(window)
    _, keys = _mask_plan(window)
    masks = _build_masks(window, keys)

    # stationary-side plane convention is (hi, lo); moving side is (lo, hi).
    xT = np.ascontiguousarray(
        x.reshape(TOK, D).T).astype(np.float32)          # [D, TOK]
    xh, xl = _split8(xT)
    x8 = _plane_pack(xh, xl, "lo", "hi")                 # moving

    inv = 1.0 / (THETA ** (np.arange(0, HD, 2, dtype=np.float64) / HD))
    freqs = np.arange(T, dtype=np.float64)[:, None] * inv[None, :]  # [T, 64]
    cosH = np.repeat(np.cos(freqs).T, 2, axis=0).astype(BF16NP)  # [128, T]
    sign = np.where(np.arange(HD) % 2 == 0, -1.0, 1.0)[:, None]
    sinH = (np.repeat(np.sin(freqs).T, 2, axis=0) * sign).astype(BF16NP)
    ident = np.eye(128).astype(BF16NP)

    in_maps = []
    for c in range(NCORES):
        wq = w_qkv[QROWS * c:QROWS * (c + 1)]
        wk = w_qkv[H * HD + HD * c: H * HD + HD * (c + 1)]
        wv = w_qkv[H * HD + G * HD + HD * c: H * HD + G * HD + HD * (c + 1)]
        W = np.ascontiguousarray(
            np.concatenate([wq, wk, wv], axis=0).T).astype(np.float32) * 64.0
        wh, wl = _split8(W)                              # [D, E]
        w8 = _plane_pack(wh, wl, "hi", "lo")             # stationary
        WO = np.ascontiguousarray(
            w_o[:, QROWS * c:QROWS * (c + 1)].T).astype(np.float32) * 64.0
        woh, wol = _split8(WO)                           # [QROWS, D]
        wo8 = _plane_pack(woh, wol, "lo", "hi")          # moving
        in_maps.append({
            "x8": x8, "w8": w8, "wo8": wo8,
            "cosH": cosH, "sinH": sinH, "masks": masks.astype(BF16NP),
            "ident": ident,
        })

    from concourse.bass_utils import run_bass_kernel_spmd
    res = run_bass_kernel_spmd(nc, in_maps, core_ids=list(range(NCORES)),
                               trace=_trace)
    global LAST_RESULTS
    LAST_RESULTS = res
    acc = res.results[0]["out"].astype(np.float32)
    for c in range(1, NCORES):
        acc = acc + res.results[c]["out"].astype(np.float32)
    # undo the 32x (oT8) * 64x (wo8) operand scaling
    return (acc / 2048.0).reshape(B, T, D)
